# revision 43
# baseline (speedup 1.0000x reference)
"""AdaLN DiT block on 8 Trainium2 NeuronCores — v7, zero collectives.

Sharding: core c owns batch b=c//2 and query-half h=c%2. Host-side the
x tokens are permuted per core so the OWN 512 tokens are always columns
0:512 of x_feat ([D, 1024] feature-major, own|other). Each core computes
LN1 + k/v projections for its FULL batch (1024 tokens) locally. q / Wo /
MLP / residuals are own-half only. No collectives at all.

v7 changes vs v2 (433.5us -> ~395us):
- LN stats matmul pairs col-group-packed: sum in (row 0, bank 0) and
  sumsq in (row 32, bank 1) of one [128, 2*TOK] psum tile. Distinct col
  groups make the pair run concurrently; distinct banks are REQUIRED —
  a start=True bank-clear from one stream lands mid-flight in a
  concurrently-streaming col-tiled matmul sharing the bank and wipes
  its first-touch bits (intermittent negative variance -> NaN).
- ln prep/apply split: per-column a=rstd / b=-mu*rstd rows broadcast
  into one [128, 2*TOK] psum tile; both halves' preps emit before the
  apply chains so the h1 sqrt doesn't head-of-line-block the Scalar
  engine FIFO in front of the projection evacuations.
- Modulates run on the Scalar engine (Identity with AP scale/bias);
  q/k/v PSUM evacuations on Scalar (Copy) — the Vector engine only
  carries the LN mult/add chain in that phase.
- Softmax exp split across engines: head-half 0 exact exp on Scalar,
  head-half 1 via a Schraudolph bf16 bit-trick on Vector (int16 bitcast
  of x*log2(e)*128 + 16248.6; ~2% relative, cancels against the
  denominator computed from the same values).
- Wo is a PSUM-accumulated GEMM after the attention loop (was SBUF f32
  partial accumulation: 64 DVE adds and a ~30us serial tail that let
  HAM re-throttle the PE into fc1). Its LN2 stats matmuls emit after
  the whole GEMM so the PE queue never blocks on the elementwise chain.
- adaLN blocks 0..15 run at the very front (PE warmup during x DMA),
  16..47 interleave into attention hp 0..5.
- Output DMAs spread across 3 queues; x2/sq epilogue split DVE/GpSimd.

PSUM budget (8 banks): tag "big" [128,1024] x2 = 4 banks (stats, ab
broadcasts, scores, fc1), tag "proj" [128,512] x2 = 2 (k/q/v psum,
mod2, Wo, fc2), tag "av" [128,512] x2 = 2 (mod1, AV, psb).
NOTE: matmul start=True clears the WHOLE psum bank; concurrent
(col-tiled) accumulation streams must therefore live in separate banks.
"""

import numpy as np

B, S, D, H, HID = 4, 1024, 1024, 16, 4096
DK = D // H  # 64
N_CORES = 8
TOK = 512    # own tokens per core
SB = 1024    # batch tokens per core (k/v computed locally)
EPS = 1e-6
KT = 8    # 128-row blocks in D
HC = 32   # 128-row blocks in HID

_cached = {}
DEBUG = False
USE_FAST_EXP = True
HAS_ROW_BIAS = False  # bv/b1 nonzero -> adds the rank-1 bias matmuls
HAS_COL_BIAS = False  # bq/bk/bo/b2/bada nonzero -> bias-add epilogues


def _build():
    import contextlib
    import concourse.bass as bass  # noqa: F401
    import concourse.tile as tile
    from concourse import bacc, mybir

    f32 = mybir.dt.float32
    bf16 = mybir.dt.bfloat16
    f8 = mybir.dt.float8e4
    PM = mybir.MatmulPerfMode.DoubleRow
    WSC = 64.0  # host-side fp8 weight scale (descaled in the evacuations)
    ASC = 16.0  # attnT fp8 activation scale (max |attn| <= max |v| ~6)
    AF = mybir.ActivationFunctionType
    OP = mybir.AluOpType

    nc = bacc.Bacc("TRN2", target_bir_lowering=False, debug=False,
                   num_devices=N_CORES)

    # ---- per-core external I/O ----
    x_feat = nc.dram_tensor("x_feat", [D, SB], bf16, kind="ExternalInput")
    condT = nc.dram_tensor("condT", [128, 8], f32, kind="ExternalInput")
    wq_t = nc.dram_tensor("wq_t", [KT, 128, D], bf16, kind="ExternalInput")
    wk_t = nc.dram_tensor("wk_t", [KT, 128, D], bf16, kind="ExternalInput")
    wo_t = nc.dram_tensor("wo_t", [KT, 128, D], bf16, kind="ExternalInput")
    wvT = nc.dram_tensor("wvT", [D, D], bf16, kind="ExternalInput")
    w1_t = nc.dram_tensor("w1_t", [KT, 128, 8 * 512], bf16, kind="ExternalInput")
    w2_t = nc.dram_tensor("w2_t", [KT, 2, 128, 2048], bf16, kind="ExternalInput")
    wada_b = nc.dram_tensor("wada_b", [48, 128, D], bf16, kind="ExternalInput")
    # packed per-partition bias columns (fp32): 0..47 bada, 48..55 bq,
    # 56..63 bk, 64..71 bo, 72..79 b2
    biasc = nc.dram_tensor("biasc", [128, 80], f32, kind="ExternalInput")
    bv_row = nc.dram_tensor("bv_row", [1, D], bf16, kind="ExternalInput")
    b1_row = nc.dram_tensor("b1_row", [1, HID], bf16, kind="ExternalInput")
    out_feat = nc.dram_tensor("out_feat", [D, TOK], f32, kind="ExternalOutput")
    if DEBUG:
        dbg_h1 = nc.dram_tensor("dbg_h1", [4, 128, SB], bf16,
                                kind="ExternalOutput")
        dbg_kT = nc.dram_tensor("dbg_kT", [KT, 128, SB], bf16,
                                kind="ExternalOutput")
        dbg_qT = nc.dram_tensor("dbg_qT", [KT, 128, TOK], bf16,
                                kind="ExternalOutput")
        dbg_v = nc.dram_tensor("dbg_v", [128, 16 * 8 * 65], bf16,
                               kind="ExternalOutput")
        dbg_mod = nc.dram_tensor("dbg_mod", [128, 48], f32,
                                 kind="ExternalOutput")

    with tile.TileContext(nc) as tc:
        ctx = contextlib.ExitStack()
        consts = ctx.enter_context(tc.tile_pool(name="consts", bufs=1))
        persist = ctx.enter_context(tc.tile_pool(name="persist", bufs=1))
        reuse = ctx.enter_context(tc.tile_pool(name="reuse", bufs=1))
        wpool = ctx.enter_context(tc.tile_pool(name="wpool", bufs=3))
        trans = ctx.enter_context(tc.tile_pool(name="trans", bufs=3))
        pT_pool = ctx.enter_context(tc.tile_pool(name="pTp", bufs=2))
        psum = ctx.enter_context(tc.tile_pool(name="psum", bufs=2, space="PSUM"))

        # ---------- constants (cond first — it gates the silu/mod path) ----
        cond_sb = consts.tile([128, 8], f32)
        nc.sync.dma_start(cond_sb[:], condT[:])
        bias_sb = consts.tile([128, 80], f32)
        nc.scalar.dma_start(bias_sb[:], biasc[:])
        if HAS_ROW_BIAS:
            bvr_sb = consts.tile([1, D], bf16)
            nc.scalar.dma_start(bvr_sb[:], bv_row[:])
            b1r_sb = consts.tile([1, HID], bf16)
            nc.scalar.dma_start(b1r_sb[:], b1_row[:])
            ones_tok = consts.tile([1, TOK], bf16)
            nc.vector.memset(ones_tok[:], 1.0)
        eps_sb = consts.tile([1, 1], f32)
        nc.vector.memset(eps_sb[:], EPS)
        ones_m = consts.tile([1, 128], bf16)
        nc.vector.memset(ones_m[:], 1.0)
        ones2 = consts.tile([65, 128], bf16)  # ones rows at partitions 0..64
        nc.vector.memset(ones2[:], 1.0)
        ones_col = consts.tile([128, 1], bf16)
        nc.vector.memset(ones_col[:], 1.0)

        def bcol(i):
            return bias_sb[:, i:i + 1]

        # ---------- adaLN modulation: silu(cond) ----------
        silu_sb = consts.tile([128, 8], bf16)
        nc.scalar.activation(silu_sb[:], cond_sb[:], AF.Silu)

        mod_sb = consts.tile([128, 48], f32)
        mod1p_sb = consts.tile([128, 48], f32)

        def emit_mod_block(ps, col, blk, first, eng=None):
            """One 128-output adaLN block: 256KB DMA + 8 stationary mms."""
            wt = wpool.tile([128, D], bf16, tag="wada", bufs=2, name="wada")
            (eng or nc.sync).dma_start(wt[:], wada_b[blk])
            for k in range(KT):
                nc.tensor.matmul(
                    ps[:, col:col + 1], lhsT=wt[:, k * 128:(k + 1) * 128],
                    rhs=silu_sb[:, k:k + 1],
                    start=(first and k == 0), stop=(k == KT - 1),
                    skip_group_check=True)

        # ---------- phase 1: x DMA + LN1 stats + gama1/beta1 ----------
        xfeat_sb = []
        for d in range(KT):
            xf = persist.tile([128, SB], bf16, tag=f"xfeat{d}", name=f"xf{d}")
            xfeat_sb.append(xf)
        for d in range(KT):
            nc.gpsimd.dma_start(xfeat_sb[d][:],
                                x_feat[d * 128:(d + 1) * 128, :])

        ps_mod = psum.tile([128, 16], f32, tag="av", bufs=2, name="ps_mod")
        # per-half stats tile: sum in row 0, sumsq in row 32 of one bank;
        # the (sum, sumsq) matmul pair runs concurrently via col-groups.
        # sum in (row 0, bank 0), sumsq in (row 32, bank 1): distinct col
        # groups make the pair concurrent, distinct banks make each stream's
        # start=True bank-clear safe against the other.
        ps_st = [psum.tile([128, 2 * TOK], f32, tag="big", name=f"ps_st{h}")
                 for h in range(2)]

        for half in range(2):
            for d in range(KT):
                xs = xfeat_sb[d][:, half * TOK:(half + 1) * TOK]
                sq = trans.tile([128, TOK], bf16, tag="lnsq", bufs=2,
                                name="lnsq")
                nc.vector.tensor_tensor(out=sq[:], in0=xs, in1=xs, op=OP.mult)
                nc.tensor.matmul(ps_st[half][0:1, 0:TOK], lhsT=ones_col[:],
                                 rhs=xs, start=(d == 0), stop=(d == KT - 1),
                                 skip_group_check=True)
                nc.tensor.matmul(ps_st[half][32:33, TOK:2 * TOK],
                                 lhsT=ones_col[:],
                                 rhs=sq[:], start=(d == 0),
                                 stop=(d == KT - 1),
                                 skip_group_check=True)
                blk = 8 * half + d  # gama1 blocks 0..7 then beta1 8..15
                emit_mod_block(ps_mod, blk, blk, first=(blk == 0))

        if HAS_COL_BIAS:
            nc.vector.tensor_tensor(out=mod_sb[:, 0:16], in0=ps_mod[:],
                                    in1=bias_sb[:, 0:16], op=OP.add)
        else:
            nc.vector.tensor_copy(out=mod_sb[:, 0:16], in_=ps_mod[:])
        nc.vector.tensor_scalar_add(mod1p_sb[:, 0:16], mod_sb[:, 0:16], 1.0)

        # ---------- LN finish helpers (prep: stats->broadcast, apply: per-d) --
        def ln_prep(ps_stat, width):
            """ps_stat rows 0(sum)/32(sumsq) -> psum [128, 2w] = [rstd | -mu*rstd]"""
            stA = trans.tile([1, 6 * width], f32, tag="lnstat", name="lnstat",
                             bufs=1)
            mu_n = stA[:, 0:width]             # -mu
            ex2 = stA[:, width:2 * width]
            var = stA[:, 2 * width:3 * width]
            tmp = stA[:, 3 * width:4 * width]  # mu^2 then std
            rstd = stA[:, 4 * width:5 * width]
            bb = stA[:, 5 * width:6 * width]   # -mu*rstd
            nc.vector.tensor_scalar(out=mu_n, in0=ps_stat[0:1, 0:width],
                                    scalar1=-1.0 / D, scalar2=None,
                                    op0=OP.mult)
            nc.vector.tensor_scalar(out=ex2,
                                    in0=ps_stat[32:33, width:2 * width],
                                    scalar1=1.0 / D, scalar2=None,
                                    op0=OP.mult)
            nc.vector.tensor_tensor(out=tmp, in0=mu_n, in1=mu_n, op=OP.mult)
            nc.vector.tensor_tensor(out=var, in0=ex2, in1=tmp, op=OP.subtract)
            nc.scalar.activation(tmp, var, AF.Sqrt, bias=eps_sb[:], scale=1.0)
            nc.vector.reciprocal_approx_fast(rstd, tmp)
            nc.vector.tensor_tensor(out=bb, in0=mu_n, in1=rstd, op=OP.mult)
            ab_bf = trans.tile([1, 2 * width], bf16, tag="lnstatbf",
                               name="lnstatbf", bufs=1)
            nc.vector.tensor_copy(out=ab_bf[:, 0:width], in_=rstd)
            nc.vector.tensor_copy(out=ab_bf[:, width:2 * width], in_=bb)
            ab = psum.tile([128, 2 * width], f32, tag="big", name="ab_bc")
            nc.tensor.matmul(ab[:, 0:width], lhsT=ones_m[:],
                             rhs=ab_bf[:, 0:width], start=True, stop=True,
                             skip_group_check=True)
            nc.tensor.matmul(ab[:, width:2 * width], lhsT=ones_m[:],
                             rhs=ab_bf[:, width:2 * width], start=True,
                             stop=True, skip_group_check=True)
            return ab

        def ln_apply(ab, width, src_cols, dst, beta_blk, gama_blk,
                     mod_on_act=False, split=False):
            absb = None
            if split:
                absb = trans.tile([128, 2 * width], bf16, tag="absb",
                                  name="absb", bufs=1)
                nc.vector.tensor_copy(out=absb[:], in_=ab[:])
            for d in range(KT):
                on_gp = split and d % 2 == 1
                eng = nc.gpsimd if on_gp else nc.vector
                A = absb if on_gp else ab
                t1 = trans.tile([128, width], bf16, tag="lnt", bufs=4,
                                name="lnt1")
                eng.tensor_tensor(out=t1[:], in0=src_cols(d),
                                  in1=A[:, 0:width], op=OP.mult)
                t2 = trans.tile([128, width], bf16, tag="lnt", bufs=4,
                                name="lnt2")
                eng.tensor_tensor(out=t2[:], in0=t1[:],
                                  in1=A[:, width:2 * width], op=OP.add)
                if mod_on_act:
                    nc.scalar.activation(
                        dst(d), t2[:], AF.Identity,
                        bias=mod_sb[:, gama_blk + d:gama_blk + d + 1],
                        scale=mod1p_sb[:, beta_blk + d:beta_blk + d + 1])
                else:
                    nc.vector.tensor_scalar(
                        out=dst(d), in0=t2[:],
                        scalar1=mod1p_sb[:, beta_blk + d:beta_blk + d + 1],
                        scalar2=mod_sb[:, gama_blk + d:gama_blk + d + 1],
                        op0=OP.mult, op1=OP.add)

        h1T = []
        for d in range(KT):
            h1T.append(reuse.tile([128, SB], bf16, tag=f"rA{d}", bufs=1,
                                  name=f"h1T{d}"))
        ab_h = [ln_prep(ps_st[h], TOK) for h in range(2)]
        for half in range(2):
            c0, c1 = half * TOK, (half + 1) * TOK
            ln_apply(ab_h[half], TOK,
                     lambda d: xfeat_sb[d][:, c0:c1],
                     lambda d: h1T[d][:, c0:c1], 8, 0, mod_on_act=True)

        if DEBUG:
            for d in range(4):
                nc.gpsimd.dma_start(dbg_h1[d], h1T[d][:])
        # ---------- projections ----------
        def evac_proj(dst, ps, bias_i):
            if HAS_COL_BIAS:
                nc.vector.tensor_scalar(out=dst, in0=ps[:],
                                        scalar1=bcol(bias_i), scalar2=None,
                                        op0=OP.add)
            else:
                nc.scalar.activation(dst, ps[:], AF.Copy)

        kT = []
        for oc in range(KT):
            kT.append(reuse.tile([128, SB], bf16, tag=f"rB{oc}", bufs=1,
                                 name=f"kT{oc}"))
        # half-outer so the own-half k projection isn't gated on the
        # other half's modulate; wk slabs stay resident across both halves
        wk_sb = []
        for oc in range(KT):
            wblk = wpool.tile([128, D], bf16, tag="wblk", bufs=8)
            nc.sync.dma_start(wblk[:], wk_t[oc])
            wk_sb.append(wblk)
        for half in range(2):
            c0, c1 = half * TOK, (half + 1) * TOK
            for oc in range(KT):
                ps = psum.tile([128, TOK], f32, tag="proj")
                for k in range(KT):
                    nc.tensor.matmul(
                        ps[:], lhsT=wk_sb[oc][:, k * 128:(k + 1) * 128],
                        rhs=h1T[k][:, c0:c1], start=(k == 0),
                        stop=(k == KT - 1))
                evac_proj(kT[oc][:, c0:c1], ps, 56 + oc)

        if DEBUG:
            for oc in range(KT):
                nc.gpsimd.dma_start(dbg_kT[oc], kT[oc][:])
        # q: own half only
        qT = []
        for oc in range(KT):
            wblk = wpool.tile([128, D], bf16, tag="wblk", bufs=8)
            nc.sync.dma_start(wblk[:], wq_t[oc])
            ps = psum.tile([128, TOK], f32, tag="proj")
            for k in range(KT):
                nc.tensor.matmul(
                    ps[:], lhsT=wblk[:, k * 128:(k + 1) * 128],
                    rhs=h1T[k][:, 0:TOK], start=(k == 0), stop=(k == KT - 1))
            qt = persist.tile([128, TOK], bf16, tag=f"qT{oc}", name=f"qT{oc}")
            evac_proj(qt[:], ps, 48 + oc)
            qT.append(qt)

        if DEBUG:
            for oc in range(KT):
                nc.gpsimd.dma_start(dbg_qT[oc], qT[oc][:])
        # v: token-major over the full batch, evacuated straight into the
        # padded per-head layout [128, head, kc, 65] (col 64 = ones).
        v_h8 = persist.tile([128, 16, 8, 65], bf16, tag="v_h8", name="v_h8")
        nc.vector.memset(v_h8[:, :, :, 64:65], 1.0)
        for fh in range(2):
            wv_tiles = []
            for k in range(KT):
                wblk = wpool.tile([128, TOK], bf16, tag="wvblk", name="wvblk",
                                  bufs=8)
                nc.sync.dma_start(
                    wblk[:], wvT[k * 128:(k + 1) * 128,
                                 fh * 512:(fh + 1) * 512])
                wv_tiles.append(wblk)
            for tb in range(8):
                ps = psum.tile([128, TOK], f32, tag="proj")
                for k in range(KT):
                    nc.tensor.matmul(
                        ps[:], lhsT=h1T[k][:, tb * 128:(tb + 1) * 128],
                        rhs=wv_tiles[k][:],
                        start=(k == 0),
                        stop=(k == KT - 1 and not HAS_ROW_BIAS),
                        skip_group_check=True)
                if HAS_ROW_BIAS:
                    nc.tensor.matmul(
                        ps[:], lhsT=ones_m[:],
                        rhs=bvr_sb[:, fh * 512:(fh + 1) * 512],
                        start=False, stop=True)
                # strided evac: [128, 512] -> heads fh*8..fh*8+7, kc=tb
                nc.scalar.activation(
                    v_h8[:, fh * 8:(fh + 1) * 8, tb, 0:64],
                    ps[:].rearrange("p (h c) -> p h c", h=8), AF.Copy)

        if DEBUG:
            nc.gpsimd.dma_start(dbg_v[:], v_h8[:].rearrange("p a b c -> p (a b c)"))
        # ---------- attention (+ interleaved adaLN blocks 32..47) ----------
        ps_mod2 = psum.tile([128, 32], f32, tag="proj", name="ps_mod2")
        MOD2_PER_HP = [6, 6, 6, 6, 6, 2, 0, 0]
        nmod2 = 0
        attnT = []
        for hp in range(KT):
            attnT.append(persist.tile([128, TOK], bf16, tag=f"attnT{hp}",
                                      name=f"attnT{hp}"))
        SCALE = 1.0 / 8.0

        # Wo weight slabs prefetched; the GEMM itself runs after the loop.
        wo_sb = []
        for dc in range(KT):
            wblk = wpool.tile([128, D], bf16, tag="wblk", bufs=8)
            nc.sync.dma_start(wblk[:], wo_t[dc])
            wo_sb.append(wblk)

        # Schraudolph fast-exp constants for the DVE path (bf16 bit trick):
        # bits16 = x*SCALE*log2(e)*128 + (127*128 - 7.41); rel err ~2% which
        # cancels between numerator and denominator of the softmax.
        EXP_MUL = SCALE * 1.4426950408889634 * 128.0
        EXP_ADD = 16256.0 - 7.41

        for hp in range(KT):
            pT_g = {}
            for g in range(4):
                ps_AB = [psum.tile([128, 2 * TOK], f32, tag="big",
                                   name=f"ps_s{hh}") for hh in range(2)]
                for i in range(2):
                    kc = 2 * g + i
                    for hh in range(2):
                        nc.tensor.matmul(
                            ps_AB[hh][:, i * TOK:(i + 1) * TOK],
                            lhsT=kT[hp][hh * 64:(hh + 1) * 64,
                                        kc * 128:(kc + 1) * 128],
                            rhs=qT[hp][hh * 64:(hh + 1) * 64, :],
                            start=True, stop=True)
                # hh=0 exact exp on ACT; hh=1 fast-exp on DVE
                pt = pT_pool.tile([128, 2 * TOK], bf16, tag="pT",
                                  name="pTg", bufs=5)
                nc.scalar.activation(out=pt[:], in_=ps_AB[0][:],
                                     func=AF.Exp, scale=SCALE)
                pT_g[(0, g)] = pt
                pti = pT_pool.tile([128, 2 * TOK], bf16, tag="pTi",
                                   name="pTi", bufs=5)
                if USE_FAST_EXP:
                    nc.vector.tensor_scalar(
                        out=pti[:].bitcast(mybir.dt.int16), in0=ps_AB[1][:],
                        scalar1=EXP_MUL, scalar2=EXP_ADD,
                        op0=OP.mult, op1=OP.add)
                else:
                    nc.scalar.activation(out=pti[:], in_=ps_AB[1][:],
                                         func=AF.Exp, scale=SCALE)
                pT_g[(1, g)] = pti
            ps_avs = []
            for hh in range(2):
                h = 2 * hp + hh
                ps_av = psum.tile([128, TOK], f32, tag="av")
                for kc in range(8):
                    nc.tensor.matmul(
                        ps_av[0:65, :], lhsT=v_h8[:, h, kc, :],
                        rhs=pT_g[(hh, kc // 2)][:, (kc % 2) * TOK:
                                                (kc % 2 + 1) * TOK],
                        start=(kc == 0), stop=(kc == 7))
                ps_avs.append(ps_av)
                # interleave adaLN blocks 16..47
                for _ in range(MOD2_PER_HP[hp] // 2):
                    if nmod2 < 32:
                        emit_mod_block(ps_mod2, nmod2, 16 + nmod2,
                                       first=(nmod2 == 0), eng=nc.gpsimd)
                        nmod2 += 1
            # per-hp softmax normalization: denominators live in row 64 of
            # each ps_av; stage both heads' reciprocals at partitions 0/64
            # (matmul rhs base must be 0/32/64)
            dn2 = trans.tile([65, 2 * TOK], f32, tag="dn", bufs=1, name="dn2")
            for hh in range(2):
                nc.vector.tensor_copy(out=dn2[64 * hh:64 * hh + 1, 0:TOK],
                                      in_=ps_avs[hh][64:65, :])
            nc.vector.reciprocal_approx_fast(dn2[:, TOK:2 * TOK],
                                             dn2[:, 0:TOK])
            rd2 = trans.tile([65, TOK], bf16, tag="rd", bufs=1, name="rd2")
            for hh in range(2):
                nc.vector.tensor_copy(
                    out=rd2[64 * hh:64 * hh + 1, :],
                    in_=dn2[64 * hh:64 * hh + 1, TOK:2 * TOK])
            for hh in range(2):
                psb = psum.tile([128, TOK], f32, tag="av", name="psb")
                nc.tensor.matmul(psb[0:64, :],
                                 lhsT=ones2[64 * hh:64 * hh + 1, 0:64],
                                 rhs=rd2[64 * hh:64 * hh + 1, :],
                                 start=True, stop=True)
                nc.vector.tensor_copy(out=attnT[hp][hh * 64:(hh + 1) * 64, :],
                                      in_=ps_avs[hh][0:64, :])
                nc.vector.tensor_tensor(
                    out=attnT[hp][hh * 64:(hh + 1) * 64, :],
                    in0=attnT[hp][hh * 64:(hh + 1) * 64, :],
                    in1=psb[0:64, :], op=OP.mult)

        # evacuate adaLN blocks 16..47 (alpha1, gama2, beta2, alpha2)
        if HAS_COL_BIAS:
            nc.vector.tensor_tensor(out=mod_sb[:, 16:48], in0=ps_mod2[:],
                                    in1=bias_sb[:, 16:48], op=OP.add)
        else:
            nc.vector.tensor_copy(out=mod_sb[:, 16:48], in_=ps_mod2[:])
        nc.vector.tensor_scalar_add(mod1p_sb[:, 16:48], mod_sb[:, 16:48], 1.0)

        if DEBUG:
            nc.gpsimd.dma_start(dbg_mod[:], mod_sb[:])
        # ---------- Wo GEMM + epilogue fused with LN2 stats ----------
        # All 64 Wo matmuls first (dense PE stream); the per-dc evac chains
        # (DVE/GpSimd) trail behind; the stats matmuls go after so they
        # don't block the PE queue on the elementwise chain.
        x2T = []
        sq2 = []
        ps_st2 = psum.tile([128, 2 * TOK], f32, tag="big", name="ps_st2")
        for dc in range(KT):
            ps_w = psum.tile([128, TOK], f32, tag="proj", name="ps_wo")
            for hp in range(KT):
                nc.tensor.matmul(ps_w[:],
                                 lhsT=wo_sb[dc][:, hp * 128:(hp + 1) * 128],
                                 rhs=attnT[hp][:], start=(hp == 0),
                                 stop=(hp == KT - 1))
            ysc = trans.tile([128, TOK], f32, tag="sc_evac", name="ysc",
                             bufs=2)
            if HAS_COL_BIAS:
                nc.vector.tensor_scalar(
                    out=ysc[:], in0=ps_w[:], scalar1=bcol(64 + dc),
                    scalar2=mod_sb[:, 16 + dc:17 + dc], op0=OP.add,
                    op1=OP.mult)
            else:
                nc.vector.tensor_scalar(
                    out=ysc[:], in0=ps_w[:],
                    scalar1=mod_sb[:, 16 + dc:17 + dc], scalar2=None,
                    op0=OP.mult)
            x2t = persist.tile([128, TOK], bf16, tag=f"x2T{dc}",
                               name=f"x2T{dc}")
            nc.vector.tensor_tensor(out=x2t[:], in0=ysc[:],
                                    in1=xfeat_sb[dc][:, 0:TOK], op=OP.add)
            x2T.append(x2t)
            sq = trans.tile([128, TOK], bf16, tag="sq2", bufs=6, name="sq2")
            nc.gpsimd.tensor_tensor(out=sq[:], in0=x2t[:], in1=x2t[:],
                                    op=OP.mult)
            sq2.append(sq)
        for dc in range(KT):
            nc.tensor.matmul(ps_st2[0:1, 0:TOK], lhsT=ones_col[:],
                             rhs=x2T[dc][:], start=(dc == 0),
                             stop=(dc == KT - 1), skip_group_check=True)
            nc.tensor.matmul(ps_st2[32:33, TOK:2 * TOK], lhsT=ones_col[:],
                             rhs=sq2[dc][:], start=(dc == 0),
                             stop=(dc == KT - 1), skip_group_check=True)

        h2T = []
        for d in range(KT):
            h2T.append(persist.tile([128, TOK], bf16, tag=f"h2T{d}",
                                    name=f"h2T{d}"))
        ab2 = ln_prep(ps_st2, TOK)
        ps_warm = psum.tile([128, TOK], f32, tag="av", name="ps_warm")
        for i in range(14):
            nc.tensor.matmul(ps_warm[0:1, :], lhsT=ones_col[:],
                             rhs=xfeat_sb[i % 8][:, 0:TOK],
                             start=True, stop=True, skip_group_check=True)
        ln_apply(ab2, TOK, lambda d: x2T[d][:], lambda d: h2T[d][:],
                 32, 24, mod_on_act=True, split=True)

        # ---------- MLP (bf16, token-local) ----------
        G_sb = []
        for g4 in range(8):  # groups of 4 HID blocks
            w1q = wpool.tile([128, 8 * 512], bf16, tag="w1q", bufs=2)
            nc.sync.dma_start(w1q[:], w1_t[g4])
            for jp in range(2):
                ps_g = psum.tile([128, 2 * TOK], f32, tag="big")
                for j2 in range(2):
                    hc = 4 * g4 + 2 * jp + j2
                    for k in range(KT):
                        nc.tensor.matmul(
                            ps_g[:, j2 * TOK:(j2 + 1) * TOK],
                            lhsT=w1q[:, k * 512 + (2 * jp + j2) * 128:
                                     k * 512 + (2 * jp + j2 + 1) * 128],
                            rhs=h2T[k][:],
                            start=(k == 0), stop=False,
                            skip_group_check=True)
                    if HAS_ROW_BIAS:
                        nc.tensor.matmul(
                            ps_g[:, j2 * TOK:(j2 + 1) * TOK],
                            lhsT=b1r_sb[:, hc * 128:(hc + 1) * 128],
                            rhs=ones_tok[:], start=False, stop=True,
                            skip_group_check=True)
                # reuse the h1T ring (idx 0..7) then the kT ring (8..15);
                # G holds the (2gi | 2gi+1) HID-chunk pair in fp8.
                gi = 2 * g4 + jp
                if gi < 8:
                    gt = reuse.tile([128, SB], bf16, tag=f"rA{gi}", bufs=1,
                                    name=f"G{gi}")
                else:
                    gt = reuse.tile([128, SB], bf16, tag=f"rB{gi - 8}", bufs=1,
                                    name=f"G{gi}")
                nc.scalar.activation(out=gt[:], in_=ps_g[:], func=AF.Gelu)
                G_sb.append(gt)

        out_q = [nc.sync, nc.scalar, nc.gpsimd, nc.scalar]
        for dc in range(KT):
            ps_z = psum.tile([128, TOK], f32, tag="proj")
            for kg2 in range(2):  # two [128, 2048] weight slabs
                wblk = wpool.tile([128, 2048], bf16, tag="w2blk", bufs=2)
                nc.sync.dma_start(wblk[:], w2_t[dc, kg2])
                for i in range(16):
                    kb = 16 * kg2 + i
                    nc.tensor.matmul(
                        ps_z[:], lhsT=wblk[:, i * 128:(i + 1) * 128],
                        rhs=G_sb[kb // 2][:, (kb % 2) * TOK:(kb % 2 + 1) * TOK],
                        start=(kb == 0), stop=(kb == HC - 1))
            zsc = trans.tile([128, TOK], f32, tag="sc_evac", name="zsc",
                             bufs=2)
            if HAS_COL_BIAS:
                nc.vector.tensor_scalar(
                    out=zsc[:], in0=ps_z[:], scalar1=bcol(72 + dc),
                    scalar2=mod_sb[:, 40 + dc:41 + dc],
                    op0=OP.add, op1=OP.mult)
            else:
                nc.vector.tensor_scalar(
                    out=zsc[:], in0=ps_z[:],
                    scalar1=mod_sb[:, 40 + dc:41 + dc], scalar2=None,
                    op0=OP.mult)
            ot = trans.tile([128, TOK], f32, tag="sc_evac", name="ot", bufs=2)
            nc.gpsimd.tensor_tensor(out=ot[:], in0=zsc[:],
                                    in1=x2T[dc][:], op=OP.add)
            out_q[dc % 4].dma_start(out_feat[dc * 128:(dc + 1) * 128, :],
                                    ot[:])

        ctx.close()

    nc.compile()
    return nc


def _pack_bias(bq, bk, bo, b2, bada):
    t = np.zeros((128, 80), np.float32)
    t[:, 0:48] = bada.reshape(48, 128).T
    t[:, 48:56] = bq.reshape(8, 128).T
    t[:, 56:64] = bk.reshape(8, 128).T
    t[:, 64:72] = bo.reshape(8, 128).T
    t[:, 72:80] = b2.reshape(8, 128).T
    return t


def _slab_oc(wT):
    """[D, D] W.T -> [8, 128, 1024]: slab[oc][p][k*128+c] = wT[k*128+p, oc*128+c]"""
    w = wT.reshape(KT, 128, KT, 128)          # [k, p, oc, c]
    return np.ascontiguousarray(w.transpose(2, 1, 0, 3).reshape(KT, 128, D))


WSC = 64.0  # fp8 weight scale (the device folds 1/WSC into the evacuations)


def _fp8(a):
    import ml_dtypes
    return np.clip(a, -240.0, 240.0).astype(ml_dtypes.float8_e4m3)


def _slab_w1(w1T):
    """[D, HID] W1.T -> [8(g4), 128(p), 8(k)*512]: slab[g4][p][k*512+c] =
    w1T[k*128+p, g4*512+c]"""
    w = w1T.reshape(KT, 128, 8, 512)          # [k, p, g4, c]
    return np.ascontiguousarray(w.transpose(2, 1, 0, 3).reshape(KT, 128, 8 * 512))


def _slab_w2(w2T):
    """[HID, D] W2.T -> [8(dc), 2(kg2), 128(p), 16(i)*128]: slab[dc,kg2,p,i*128+c]
    = w2T[(16*kg2+i)*128+p, dc*128+c]"""
    w = w2T.reshape(2, 16, 128, KT, 128)      # [kg2, i, p, dc, c]
    return np.ascontiguousarray(
        w.transpose(3, 0, 2, 1, 4).reshape(KT, 2, 128, 2048))


def _slab_wada(wadaT):
    """[D, 6D] Wada.T -> [48, 128, 1024]: slab[blk][p][k*128+c] =
    wadaT[k*128+p, blk*128+c]"""
    w = wadaT.reshape(KT, 128, 48, 128)       # [k, p, blk, c]
    return np.ascontiguousarray(w.transpose(2, 1, 0, 3).reshape(48, 128, D))


def kernel(x, cond, Wq, bq, Wk, bk, Wv, bv, Wo, bo, W1, b1, W2, b2, Wada, bada):
    import ml_dtypes
    from concourse.bass_utils import run_bass_kernel_spmd

    bf = ml_dtypes.bfloat16
    global HAS_ROW_BIAS, HAS_COL_BIAS
    if "nc" not in _cached:
        HAS_ROW_BIAS = bool(np.any(np.asarray(bv)) or np.any(np.asarray(b1)))
        HAS_COL_BIAS = bool(np.any(np.asarray(bq)) or np.any(np.asarray(bk))
                            or np.any(np.asarray(bo)) or np.any(np.asarray(b2))
                            or np.any(np.asarray(bada)))
        _cached["nc"] = _build()
    nc = _cached["nc"]

    x = np.asarray(x, np.float32)
    cond = np.asarray(cond, np.float32)
    to_bf_T = lambda w: np.ascontiguousarray(
        np.asarray(w, np.float32).T).astype(bf)
    wq_t = _slab_oc(np.asarray(Wq, np.float32).T).astype(bf)
    wk_t = _slab_oc(np.asarray(Wk, np.float32).T).astype(bf)
    wo_t = _slab_oc(np.asarray(Wo, np.float32).T).astype(bf)
    wvT = to_bf_T(Wv)
    w1_t = _slab_w1(np.asarray(W1, np.float32).T).astype(bf)
    w2_t = _slab_w2(np.asarray(W2, np.float32).T).astype(bf)
    wada_b = _slab_wada(np.asarray(Wada, np.float32).T).astype(bf)
    biasc = _pack_bias(np.asarray(bq, np.float32), np.asarray(bk, np.float32),
                       np.asarray(bo, np.float32),
                       np.asarray(b2, np.float32),
                       np.asarray(bada, np.float32))
    bv_row = np.asarray(bv, np.float32).reshape(1, D).astype(bf)

    in_maps = []
    for c in range(N_CORES):
        b, h = c // 2, c % 2
        # own 512 tokens first, then the other half (token-permuted batch)
        xs = np.concatenate([x[b, h * TOK:(h + 1) * TOK, :],
                             x[b, (1 - h) * TOK:(2 - h) * TOK, :]], axis=0)
        in_maps.append({
            "x_feat": np.ascontiguousarray(xs.T).astype(bf),
            "condT": np.ascontiguousarray(cond[b, 0].reshape(8, 128).T),
            "wq_t": wq_t, "wk_t": wk_t, "wo_t": wo_t, "wvT": wvT,
            "w1_t": w1_t, "w2_t": w2_t, "wada_b": wada_b,
            "biasc": biasc, "bv_row": bv_row,
            "b1_row": np.asarray(b1, np.float32).reshape(1, HID).astype(bf),
        })

    _cached["in_maps"] = in_maps
    res = run_bass_kernel_spmd(nc, in_maps, core_ids=list(range(N_CORES)))
    _cached["results"] = res.results
    out = np.empty((B, S, D), np.float32)
    for c in range(N_CORES):
        b, h = c // 2, c % 2
        out[b, h * TOK:(h + 1) * TOK, :] = res.results[c]["out_feat"].T
    return out


# revision 44
# speedup vs baseline: 1.0021x; 1.0021x over previous
"""AdaLN DiT block on 8 Trainium2 NeuronCores — v7, zero collectives.

Sharding: core c owns batch b=c//2 and query-half h=c%2. Host-side the
x tokens are permuted per core so the OWN 512 tokens are always columns
0:512 of x_feat ([D, 1024] feature-major, own|other). Each core computes
LN1 + k/v projections for its FULL batch (1024 tokens) locally. q / Wo /
MLP / residuals are own-half only. No collectives at all.

v7 changes vs v2 (433.5us -> ~395us):
- LN stats matmul pairs col-group-packed: sum in (row 0, bank 0) and
  sumsq in (row 32, bank 1) of one [128, 2*TOK] psum tile. Distinct col
  groups make the pair run concurrently; distinct banks are REQUIRED —
  a start=True bank-clear from one stream lands mid-flight in a
  concurrently-streaming col-tiled matmul sharing the bank and wipes
  its first-touch bits (intermittent negative variance -> NaN).
- ln prep/apply split: per-column a=rstd / b=-mu*rstd rows broadcast
  into one [128, 2*TOK] psum tile; both halves' preps emit before the
  apply chains so the h1 sqrt doesn't head-of-line-block the Scalar
  engine FIFO in front of the projection evacuations.
- Modulates run on the Scalar engine (Identity with AP scale/bias);
  q/k/v PSUM evacuations on Scalar (Copy) — the Vector engine only
  carries the LN mult/add chain in that phase.
- Softmax exp split across engines: head-half 0 exact exp on Scalar,
  head-half 1 via a Schraudolph bf16 bit-trick on Vector (int16 bitcast
  of x*log2(e)*128 + 16248.6; ~2% relative, cancels against the
  denominator computed from the same values).
- Wo is a PSUM-accumulated GEMM after the attention loop (was SBUF f32
  partial accumulation: 64 DVE adds and a ~30us serial tail that let
  HAM re-throttle the PE into fc1). Its LN2 stats matmuls emit after
  the whole GEMM so the PE queue never blocks on the elementwise chain.
- adaLN blocks 0..15 run at the very front (PE warmup during x DMA),
  16..47 interleave into attention hp 0..5.
- Output DMAs spread across 3 queues; x2/sq epilogue split DVE/GpSimd.

PSUM budget (8 banks): tag "big" [128,1024] x2 = 4 banks (stats, ab
broadcasts, scores, fc1), tag "proj" [128,512] x2 = 2 (k/q/v psum,
mod2, Wo, fc2), tag "av" [128,512] x2 = 2 (mod1, AV, psb).
NOTE: matmul start=True clears the WHOLE psum bank; concurrent
(col-tiled) accumulation streams must therefore live in separate banks.
"""

import numpy as np

B, S, D, H, HID = 4, 1024, 1024, 16, 4096
DK = D // H  # 64
N_CORES = 8
TOK = 512    # own tokens per core
SB = 1024    # batch tokens per core (k/v computed locally)
EPS = 1e-6
KT = 8    # 128-row blocks in D
HC = 32   # 128-row blocks in HID

_cached = {}
DEBUG = False
USE_FAST_EXP = True
HAS_ROW_BIAS = False  # bv/b1 nonzero -> adds the rank-1 bias matmuls
HAS_COL_BIAS = False  # bq/bk/bo/b2/bada nonzero -> bias-add epilogues


def _build():
    import contextlib
    import concourse.bass as bass  # noqa: F401
    import concourse.tile as tile
    from concourse import bacc, mybir

    f32 = mybir.dt.float32
    bf16 = mybir.dt.bfloat16
    f8 = mybir.dt.float8e4
    PM = mybir.MatmulPerfMode.DoubleRow
    WSC = 64.0  # host-side fp8 weight scale (descaled in the evacuations)
    ASC = 16.0  # attnT fp8 activation scale (max |attn| <= max |v| ~6)
    AF = mybir.ActivationFunctionType
    OP = mybir.AluOpType

    nc = bacc.Bacc("TRN2", target_bir_lowering=False, debug=False,
                   num_devices=N_CORES)

    # ---- per-core external I/O ----
    x_feat = nc.dram_tensor("x_feat", [D, SB], bf16, kind="ExternalInput")
    condT = nc.dram_tensor("condT", [128, 8], f32, kind="ExternalInput")
    wq_t = nc.dram_tensor("wq_t", [KT, 128, D], bf16, kind="ExternalInput")
    wk_t = nc.dram_tensor("wk_t", [KT, 128, D], bf16, kind="ExternalInput")
    wo_t = nc.dram_tensor("wo_t", [KT, 128, D], bf16, kind="ExternalInput")
    wvT = nc.dram_tensor("wvT", [D, D], bf16, kind="ExternalInput")
    w1_t = nc.dram_tensor("w1_t", [KT, 128, 8 * 512], bf16, kind="ExternalInput")
    w2_t = nc.dram_tensor("w2_t", [KT, 2, 128, 2048], bf16, kind="ExternalInput")
    wada_b = nc.dram_tensor("wada_b", [48, 128, D], bf16, kind="ExternalInput")
    # packed per-partition bias columns (fp32): 0..47 bada, 48..55 bq,
    # 56..63 bk, 64..71 bo, 72..79 b2
    biasc = nc.dram_tensor("biasc", [128, 80], f32, kind="ExternalInput")
    bv_row = nc.dram_tensor("bv_row", [1, D], bf16, kind="ExternalInput")
    b1_row = nc.dram_tensor("b1_row", [1, HID], bf16, kind="ExternalInput")
    out_feat = nc.dram_tensor("out_feat", [D, TOK], f32, kind="ExternalOutput")
    if DEBUG:
        dbg_h1 = nc.dram_tensor("dbg_h1", [4, 128, SB], bf16,
                                kind="ExternalOutput")
        dbg_kT = nc.dram_tensor("dbg_kT", [KT, 128, SB], bf16,
                                kind="ExternalOutput")
        dbg_qT = nc.dram_tensor("dbg_qT", [KT, 128, TOK], bf16,
                                kind="ExternalOutput")
        dbg_v = nc.dram_tensor("dbg_v", [128, 16 * 8 * 65], bf16,
                               kind="ExternalOutput")
        dbg_mod = nc.dram_tensor("dbg_mod", [128, 48], f32,
                                 kind="ExternalOutput")

    with tile.TileContext(nc) as tc:
        ctx = contextlib.ExitStack()
        consts = ctx.enter_context(tc.tile_pool(name="consts", bufs=1))
        persist = ctx.enter_context(tc.tile_pool(name="persist", bufs=1))
        reuse = ctx.enter_context(tc.tile_pool(name="reuse", bufs=1))
        wpool = ctx.enter_context(tc.tile_pool(name="wpool", bufs=3))
        trans = ctx.enter_context(tc.tile_pool(name="trans", bufs=3))
        pT_pool = ctx.enter_context(tc.tile_pool(name="pTp", bufs=2))
        psum = ctx.enter_context(tc.tile_pool(name="psum", bufs=2, space="PSUM"))

        # ---------- constants (cond first — it gates the silu/mod path) ----
        cond_sb = consts.tile([128, 8], f32)
        nc.sync.dma_start(cond_sb[:], condT[:])
        bias_sb = consts.tile([128, 80], f32)
        nc.scalar.dma_start(bias_sb[:], biasc[:])
        if HAS_ROW_BIAS:
            bvr_sb = consts.tile([1, D], bf16)
            nc.scalar.dma_start(bvr_sb[:], bv_row[:])
            b1r_sb = consts.tile([1, HID], bf16)
            nc.scalar.dma_start(b1r_sb[:], b1_row[:])
            ones_tok = consts.tile([1, TOK], bf16)
            nc.vector.memset(ones_tok[:], 1.0)
        eps_sb = consts.tile([1, 1], f32)
        nc.vector.memset(eps_sb[:], EPS)
        ones_m = consts.tile([1, 128], bf16)
        nc.vector.memset(ones_m[:], 1.0)
        ones2 = consts.tile([65, 128], bf16)  # ones rows at partitions 0..64
        nc.vector.memset(ones2[:], 1.0)
        ones_col = consts.tile([128, 1], bf16)
        nc.vector.memset(ones_col[:], 1.0)

        def bcol(i):
            return bias_sb[:, i:i + 1]

        # ---------- adaLN modulation: silu(cond) ----------
        silu_sb = consts.tile([128, 8], bf16)
        nc.scalar.activation(silu_sb[:], cond_sb[:], AF.Silu)

        mod_sb = consts.tile([128, 48], f32)
        mod1p_sb = consts.tile([128, 48], f32)

        def emit_mod_block(ps, col, blk, first, eng=None):
            """One 128-output adaLN block: 256KB DMA + 8 stationary mms."""
            wt = wpool.tile([128, D], bf16, tag="wada", bufs=2, name="wada")
            (eng or nc.sync).dma_start(wt[:], wada_b[blk])
            for k in range(KT):
                nc.tensor.matmul(
                    ps[:, col:col + 1], lhsT=wt[:, k * 128:(k + 1) * 128],
                    rhs=silu_sb[:, k:k + 1],
                    start=(first and k == 0), stop=(k == KT - 1),
                    skip_group_check=True)

        # ---------- phase 1: x DMA + LN1 stats + gama1/beta1 ----------
        xfeat_sb = []
        for d in range(KT):
            xf = persist.tile([128, SB], bf16, tag=f"xfeat{d}", name=f"xf{d}")
            xfeat_sb.append(xf)
        # own-half columns first so stats h0 can start earliest
        for half in range(2):
            for d in range(KT):
                nc.gpsimd.dma_start(
                    xfeat_sb[d][:, half * TOK:(half + 1) * TOK],
                    x_feat[d * 128:(d + 1) * 128, half * TOK:(half + 1) * TOK])

        ps_mod = psum.tile([128, 16], f32, tag="av", bufs=2, name="ps_mod")
        # per-half stats tile: sum in row 0, sumsq in row 32 of one bank;
        # the (sum, sumsq) matmul pair runs concurrently via col-groups.
        # sum in (row 0, bank 0), sumsq in (row 32, bank 1): distinct col
        # groups make the pair concurrent, distinct banks make each stream's
        # start=True bank-clear safe against the other.
        ps_st = [psum.tile([128, 2 * TOK], f32, tag="big", name=f"ps_st{h}")
                 for h in range(2)]

        for half in range(2):
            for d in range(KT):
                xs = xfeat_sb[d][:, half * TOK:(half + 1) * TOK]
                sq = trans.tile([128, TOK], bf16, tag="lnsq", bufs=2,
                                name="lnsq")
                nc.vector.tensor_tensor(out=sq[:], in0=xs, in1=xs, op=OP.mult)
                nc.tensor.matmul(ps_st[half][0:1, 0:TOK], lhsT=ones_col[:],
                                 rhs=xs, start=(d == 0), stop=(d == KT - 1),
                                 skip_group_check=True)
                nc.tensor.matmul(ps_st[half][32:33, TOK:2 * TOK],
                                 lhsT=ones_col[:],
                                 rhs=sq[:], start=(d == 0),
                                 stop=(d == KT - 1),
                                 skip_group_check=True)
                blk = 8 * half + d  # gama1 blocks 0..7 then beta1 8..15
                emit_mod_block(ps_mod, blk, blk, first=(blk == 0))

        if HAS_COL_BIAS:
            nc.vector.tensor_tensor(out=mod_sb[:, 0:16], in0=ps_mod[:],
                                    in1=bias_sb[:, 0:16], op=OP.add)
        else:
            nc.vector.tensor_copy(out=mod_sb[:, 0:16], in_=ps_mod[:])
        nc.vector.tensor_scalar_add(mod1p_sb[:, 0:16], mod_sb[:, 0:16], 1.0)

        # ---------- LN finish helpers (prep: stats->broadcast, apply: per-d) --
        def ln_prep(ps_stat, width):
            """ps_stat rows 0(sum)/32(sumsq) -> psum [128, 2w] = [rstd | -mu*rstd]"""
            stA = trans.tile([1, 6 * width], f32, tag="lnstat", name="lnstat",
                             bufs=1)
            mu_n = stA[:, 0:width]             # -mu
            ex2 = stA[:, width:2 * width]
            var = stA[:, 2 * width:3 * width]
            tmp = stA[:, 3 * width:4 * width]  # mu^2 then std
            rstd = stA[:, 4 * width:5 * width]
            bb = stA[:, 5 * width:6 * width]   # -mu*rstd
            nc.vector.tensor_scalar(out=mu_n, in0=ps_stat[0:1, 0:width],
                                    scalar1=-1.0 / D, scalar2=None,
                                    op0=OP.mult)
            nc.vector.tensor_scalar(out=ex2,
                                    in0=ps_stat[32:33, width:2 * width],
                                    scalar1=1.0 / D, scalar2=None,
                                    op0=OP.mult)
            nc.vector.tensor_tensor(out=tmp, in0=mu_n, in1=mu_n, op=OP.mult)
            nc.vector.tensor_tensor(out=var, in0=ex2, in1=tmp, op=OP.subtract)
            nc.scalar.activation(tmp, var, AF.Sqrt, bias=eps_sb[:], scale=1.0)
            nc.vector.reciprocal_approx_fast(rstd, tmp)
            nc.vector.tensor_tensor(out=bb, in0=mu_n, in1=rstd, op=OP.mult)
            ab_bf = trans.tile([1, 2 * width], bf16, tag="lnstatbf",
                               name="lnstatbf", bufs=1)
            nc.vector.tensor_copy(out=ab_bf[:, 0:width], in_=rstd)
            nc.vector.tensor_copy(out=ab_bf[:, width:2 * width], in_=bb)
            ab = psum.tile([128, 2 * width], f32, tag="big", name="ab_bc")
            nc.tensor.matmul(ab[:, 0:width], lhsT=ones_m[:],
                             rhs=ab_bf[:, 0:width], start=True, stop=True,
                             skip_group_check=True)
            nc.tensor.matmul(ab[:, width:2 * width], lhsT=ones_m[:],
                             rhs=ab_bf[:, width:2 * width], start=True,
                             stop=True, skip_group_check=True)
            return ab

        def ln_apply(ab, width, src_cols, dst, beta_blk, gama_blk,
                     mod_on_act=False, split=False):
            absb = None
            if split:
                absb = trans.tile([128, 2 * width], bf16, tag="absb",
                                  name="absb", bufs=1)
                nc.vector.tensor_copy(out=absb[:], in_=ab[:])
            for d in range(KT):
                on_gp = split and d % 2 == 1
                eng = nc.gpsimd if on_gp else nc.vector
                A = absb if on_gp else ab
                t1 = trans.tile([128, width], bf16, tag="lnt", bufs=4,
                                name="lnt1")
                eng.tensor_tensor(out=t1[:], in0=src_cols(d),
                                  in1=A[:, 0:width], op=OP.mult)
                t2 = trans.tile([128, width], bf16, tag="lnt", bufs=4,
                                name="lnt2")
                eng.tensor_tensor(out=t2[:], in0=t1[:],
                                  in1=A[:, width:2 * width], op=OP.add)
                if mod_on_act:
                    nc.scalar.activation(
                        dst(d), t2[:], AF.Identity,
                        bias=mod_sb[:, gama_blk + d:gama_blk + d + 1],
                        scale=mod1p_sb[:, beta_blk + d:beta_blk + d + 1])
                else:
                    nc.vector.tensor_scalar(
                        out=dst(d), in0=t2[:],
                        scalar1=mod1p_sb[:, beta_blk + d:beta_blk + d + 1],
                        scalar2=mod_sb[:, gama_blk + d:gama_blk + d + 1],
                        op0=OP.mult, op1=OP.add)

        h1T = []
        for d in range(KT):
            h1T.append(reuse.tile([128, SB], bf16, tag=f"rA{d}", bufs=1,
                                  name=f"h1T{d}"))
        ab_h = [ln_prep(ps_st[h], TOK) for h in range(2)]
        for half in range(2):
            c0, c1 = half * TOK, (half + 1) * TOK
            ln_apply(ab_h[half], TOK,
                     lambda d: xfeat_sb[d][:, c0:c1],
                     lambda d: h1T[d][:, c0:c1], 8, 0, mod_on_act=True)

        if DEBUG:
            for d in range(4):
                nc.gpsimd.dma_start(dbg_h1[d], h1T[d][:])
        # ---------- projections ----------
        def evac_proj(dst, ps, bias_i):
            if HAS_COL_BIAS:
                nc.vector.tensor_scalar(out=dst, in0=ps[:],
                                        scalar1=bcol(bias_i), scalar2=None,
                                        op0=OP.add)
            else:
                nc.scalar.activation(dst, ps[:], AF.Copy)

        kT = []
        for oc in range(KT):
            kT.append(reuse.tile([128, SB], bf16, tag=f"rB{oc}", bufs=1,
                                 name=f"kT{oc}"))
        # half-outer so the own-half k projection isn't gated on the
        # other half's modulate; wk slabs stay resident across both halves
        wk_sb = []
        for oc in range(KT):
            wblk = wpool.tile([128, D], bf16, tag="wblk", bufs=8)
            nc.sync.dma_start(wblk[:], wk_t[oc])
            wk_sb.append(wblk)
        for half in range(2):
            c0, c1 = half * TOK, (half + 1) * TOK
            for oc in range(KT):
                ps = psum.tile([128, TOK], f32, tag="proj")
                for k in range(KT):
                    nc.tensor.matmul(
                        ps[:], lhsT=wk_sb[oc][:, k * 128:(k + 1) * 128],
                        rhs=h1T[k][:, c0:c1], start=(k == 0),
                        stop=(k == KT - 1))
                evac_proj(kT[oc][:, c0:c1], ps, 56 + oc)

        if DEBUG:
            for oc in range(KT):
                nc.gpsimd.dma_start(dbg_kT[oc], kT[oc][:])
        # q: own half only
        qT = []
        for oc in range(KT):
            wblk = wpool.tile([128, D], bf16, tag="wblk", bufs=8)
            nc.sync.dma_start(wblk[:], wq_t[oc])
            ps = psum.tile([128, TOK], f32, tag="proj")
            for k in range(KT):
                nc.tensor.matmul(
                    ps[:], lhsT=wblk[:, k * 128:(k + 1) * 128],
                    rhs=h1T[k][:, 0:TOK], start=(k == 0), stop=(k == KT - 1))
            qt = persist.tile([128, TOK], bf16, tag=f"qT{oc}", name=f"qT{oc}")
            evac_proj(qt[:], ps, 48 + oc)
            qT.append(qt)

        if DEBUG:
            for oc in range(KT):
                nc.gpsimd.dma_start(dbg_qT[oc], qT[oc][:])
        # v: token-major over the full batch, evacuated straight into the
        # padded per-head layout [128, head, kc, 65] (col 64 = ones).
        v_h8 = persist.tile([128, 16, 8, 65], bf16, tag="v_h8", name="v_h8")
        nc.vector.memset(v_h8[:, :, :, 64:65], 1.0)
        for fh in range(2):
            wv_tiles = []
            for k in range(KT):
                wblk = wpool.tile([128, TOK], bf16, tag="wvblk", name="wvblk",
                                  bufs=8)
                nc.sync.dma_start(
                    wblk[:], wvT[k * 128:(k + 1) * 128,
                                 fh * 512:(fh + 1) * 512])
                wv_tiles.append(wblk)
            for tb in range(8):
                ps = psum.tile([128, TOK], f32, tag="proj")
                for k in range(KT):
                    nc.tensor.matmul(
                        ps[:], lhsT=h1T[k][:, tb * 128:(tb + 1) * 128],
                        rhs=wv_tiles[k][:],
                        start=(k == 0),
                        stop=(k == KT - 1 and not HAS_ROW_BIAS),
                        skip_group_check=True)
                if HAS_ROW_BIAS:
                    nc.tensor.matmul(
                        ps[:], lhsT=ones_m[:],
                        rhs=bvr_sb[:, fh * 512:(fh + 1) * 512],
                        start=False, stop=True)
                # strided evac: [128, 512] -> heads fh*8..fh*8+7, kc=tb
                nc.scalar.activation(
                    v_h8[:, fh * 8:(fh + 1) * 8, tb, 0:64],
                    ps[:].rearrange("p (h c) -> p h c", h=8), AF.Copy)

        if DEBUG:
            nc.gpsimd.dma_start(dbg_v[:], v_h8[:].rearrange("p a b c -> p (a b c)"))
        # ---------- attention (+ interleaved adaLN blocks 32..47) ----------
        ps_mod2 = psum.tile([128, 32], f32, tag="proj", name="ps_mod2")
        MOD2_PER_HP = [6, 6, 6, 6, 6, 2, 0, 0]
        nmod2 = 0
        attnT = []
        for hp in range(KT):
            attnT.append(persist.tile([128, TOK], bf16, tag=f"attnT{hp}",
                                      name=f"attnT{hp}"))
        SCALE = 1.0 / 8.0

        # Wo weight slabs prefetched; the GEMM itself runs after the loop.
        wo_sb = []
        for dc in range(KT):
            wblk = wpool.tile([128, D], bf16, tag="wblk", bufs=8)
            nc.sync.dma_start(wblk[:], wo_t[dc])
            wo_sb.append(wblk)

        # Schraudolph fast-exp constants for the DVE path (bf16 bit trick):
        # bits16 = x*SCALE*log2(e)*128 + (127*128 - 7.41); rel err ~2% which
        # cancels between numerator and denominator of the softmax.
        EXP_MUL = SCALE * 1.4426950408889634 * 128.0
        EXP_ADD = 16256.0 - 7.41

        for hp in range(KT):
            pT_g = {}
            for g in range(4):
                ps_AB = [psum.tile([128, 2 * TOK], f32, tag="big",
                                   name=f"ps_s{hh}") for hh in range(2)]
                for i in range(2):
                    kc = 2 * g + i
                    for hh in range(2):
                        nc.tensor.matmul(
                            ps_AB[hh][:, i * TOK:(i + 1) * TOK],
                            lhsT=kT[hp][hh * 64:(hh + 1) * 64,
                                        kc * 128:(kc + 1) * 128],
                            rhs=qT[hp][hh * 64:(hh + 1) * 64, :],
                            start=True, stop=True)
                # hh=0 exact exp on ACT; hh=1 fast-exp on DVE
                pt = pT_pool.tile([128, 2 * TOK], bf16, tag="pT",
                                  name="pTg", bufs=5)
                nc.scalar.activation(out=pt[:], in_=ps_AB[0][:],
                                     func=AF.Exp, scale=SCALE)
                pT_g[(0, g)] = pt
                pti = pT_pool.tile([128, 2 * TOK], bf16, tag="pTi",
                                   name="pTi", bufs=5)
                if USE_FAST_EXP:
                    nc.vector.tensor_scalar(
                        out=pti[:].bitcast(mybir.dt.int16), in0=ps_AB[1][:],
                        scalar1=EXP_MUL, scalar2=EXP_ADD,
                        op0=OP.mult, op1=OP.add)
                else:
                    nc.scalar.activation(out=pti[:], in_=ps_AB[1][:],
                                         func=AF.Exp, scale=SCALE)
                pT_g[(1, g)] = pti
            ps_avs = []
            for hh in range(2):
                h = 2 * hp + hh
                ps_av = psum.tile([128, TOK], f32, tag="av")
                for kc in range(8):
                    nc.tensor.matmul(
                        ps_av[0:65, :], lhsT=v_h8[:, h, kc, :],
                        rhs=pT_g[(hh, kc // 2)][:, (kc % 2) * TOK:
                                                (kc % 2 + 1) * TOK],
                        start=(kc == 0), stop=(kc == 7))
                ps_avs.append(ps_av)
                # interleave adaLN blocks 16..47
                for _ in range(MOD2_PER_HP[hp] // 2):
                    if nmod2 < 32:
                        emit_mod_block(ps_mod2, nmod2, 16 + nmod2,
                                       first=(nmod2 == 0), eng=nc.gpsimd)
                        nmod2 += 1
            # per-hp softmax normalization: denominators live in row 64 of
            # each ps_av; stage both heads' reciprocals at partitions 0/64
            # (matmul rhs base must be 0/32/64)
            dn2 = trans.tile([65, 2 * TOK], f32, tag="dn", bufs=1, name="dn2")
            for hh in range(2):
                nc.vector.tensor_copy(out=dn2[64 * hh:64 * hh + 1, 0:TOK],
                                      in_=ps_avs[hh][64:65, :])
            nc.vector.reciprocal_approx_fast(dn2[:, TOK:2 * TOK],
                                             dn2[:, 0:TOK])
            rd2 = trans.tile([65, TOK], bf16, tag="rd", bufs=1, name="rd2")
            for hh in range(2):
                nc.vector.tensor_copy(
                    out=rd2[64 * hh:64 * hh + 1, :],
                    in_=dn2[64 * hh:64 * hh + 1, TOK:2 * TOK])
            for hh in range(2):
                psb = psum.tile([128, TOK], f32, tag="av", name="psb")
                nc.tensor.matmul(psb[0:64, :],
                                 lhsT=ones2[64 * hh:64 * hh + 1, 0:64],
                                 rhs=rd2[64 * hh:64 * hh + 1, :],
                                 start=True, stop=True)
                nc.vector.tensor_copy(out=attnT[hp][hh * 64:(hh + 1) * 64, :],
                                      in_=ps_avs[hh][0:64, :])
                nc.vector.tensor_tensor(
                    out=attnT[hp][hh * 64:(hh + 1) * 64, :],
                    in0=attnT[hp][hh * 64:(hh + 1) * 64, :],
                    in1=psb[0:64, :], op=OP.mult)

        # evacuate adaLN blocks 16..47 (alpha1, gama2, beta2, alpha2)
        if HAS_COL_BIAS:
            nc.vector.tensor_tensor(out=mod_sb[:, 16:48], in0=ps_mod2[:],
                                    in1=bias_sb[:, 16:48], op=OP.add)
        else:
            nc.vector.tensor_copy(out=mod_sb[:, 16:48], in_=ps_mod2[:])
        nc.vector.tensor_scalar_add(mod1p_sb[:, 16:48], mod_sb[:, 16:48], 1.0)

        if DEBUG:
            nc.gpsimd.dma_start(dbg_mod[:], mod_sb[:])
        # ---------- Wo GEMM + epilogue fused with LN2 stats ----------
        # All 64 Wo matmuls first (dense PE stream); the per-dc evac chains
        # (DVE/GpSimd) trail behind; the stats matmuls go after so they
        # don't block the PE queue on the elementwise chain.
        x2T = []
        sq2 = []
        ps_st2 = psum.tile([128, 2 * TOK], f32, tag="big", name="ps_st2")
        for dc in range(KT):
            ps_w = psum.tile([128, TOK], f32, tag="proj", name="ps_wo")
            for hp in range(KT):
                nc.tensor.matmul(ps_w[:],
                                 lhsT=wo_sb[dc][:, hp * 128:(hp + 1) * 128],
                                 rhs=attnT[hp][:], start=(hp == 0),
                                 stop=(hp == KT - 1))
            ysc = trans.tile([128, TOK], f32, tag="sc_evac", name="ysc",
                             bufs=2)
            if HAS_COL_BIAS:
                nc.vector.tensor_scalar(
                    out=ysc[:], in0=ps_w[:], scalar1=bcol(64 + dc),
                    scalar2=mod_sb[:, 16 + dc:17 + dc], op0=OP.add,
                    op1=OP.mult)
            else:
                nc.vector.tensor_scalar(
                    out=ysc[:], in0=ps_w[:],
                    scalar1=mod_sb[:, 16 + dc:17 + dc], scalar2=None,
                    op0=OP.mult)
            x2t = persist.tile([128, TOK], bf16, tag=f"x2T{dc}",
                               name=f"x2T{dc}")
            nc.vector.tensor_tensor(out=x2t[:], in0=ysc[:],
                                    in1=xfeat_sb[dc][:, 0:TOK], op=OP.add)
            x2T.append(x2t)
            sq = trans.tile([128, TOK], bf16, tag="sq2", bufs=6, name="sq2")
            nc.gpsimd.tensor_tensor(out=sq[:], in0=x2t[:], in1=x2t[:],
                                    op=OP.mult)
            sq2.append(sq)
        for dc in range(KT):
            nc.tensor.matmul(ps_st2[0:1, 0:TOK], lhsT=ones_col[:],
                             rhs=x2T[dc][:], start=(dc == 0),
                             stop=(dc == KT - 1), skip_group_check=True)
            nc.tensor.matmul(ps_st2[32:33, TOK:2 * TOK], lhsT=ones_col[:],
                             rhs=sq2[dc][:], start=(dc == 0),
                             stop=(dc == KT - 1), skip_group_check=True)

        h2T = []
        for d in range(KT):
            h2T.append(persist.tile([128, TOK], bf16, tag=f"h2T{d}",
                                    name=f"h2T{d}"))
        ab2 = ln_prep(ps_st2, TOK)
        ps_warm = psum.tile([128, TOK], f32, tag="av", name="ps_warm")
        for i in range(14):
            nc.tensor.matmul(ps_warm[0:1, :], lhsT=ones_col[:],
                             rhs=xfeat_sb[i % 8][:, 0:TOK],
                             start=True, stop=True, skip_group_check=True)
        ln_apply(ab2, TOK, lambda d: x2T[d][:], lambda d: h2T[d][:],
                 32, 24, mod_on_act=True, split=True)

        # ---------- MLP (bf16, token-local) ----------
        G_sb = []
        for g4 in range(8):  # groups of 4 HID blocks
            w1q = wpool.tile([128, 8 * 512], bf16, tag="w1q", bufs=2)
            nc.sync.dma_start(w1q[:], w1_t[g4])
            for jp in range(2):
                ps_g = psum.tile([128, 2 * TOK], f32, tag="big")
                for j2 in range(2):
                    hc = 4 * g4 + 2 * jp + j2
                    for k in range(KT):
                        nc.tensor.matmul(
                            ps_g[:, j2 * TOK:(j2 + 1) * TOK],
                            lhsT=w1q[:, k * 512 + (2 * jp + j2) * 128:
                                     k * 512 + (2 * jp + j2 + 1) * 128],
                            rhs=h2T[k][:],
                            start=(k == 0), stop=False,
                            skip_group_check=True)
                    if HAS_ROW_BIAS:
                        nc.tensor.matmul(
                            ps_g[:, j2 * TOK:(j2 + 1) * TOK],
                            lhsT=b1r_sb[:, hc * 128:(hc + 1) * 128],
                            rhs=ones_tok[:], start=False, stop=True,
                            skip_group_check=True)
                # reuse the h1T ring (idx 0..7) then the kT ring (8..15);
                # G holds the (2gi | 2gi+1) HID-chunk pair in fp8.
                gi = 2 * g4 + jp
                if gi < 8:
                    gt = reuse.tile([128, SB], bf16, tag=f"rA{gi}", bufs=1,
                                    name=f"G{gi}")
                else:
                    gt = reuse.tile([128, SB], bf16, tag=f"rB{gi - 8}", bufs=1,
                                    name=f"G{gi}")
                nc.scalar.activation(out=gt[:], in_=ps_g[:], func=AF.Gelu)
                G_sb.append(gt)

        out_q = [nc.sync, nc.scalar, nc.gpsimd, nc.scalar]
        for dc in range(KT):
            ps_z = psum.tile([128, TOK], f32, tag="proj")
            for kg2 in range(2):  # two [128, 2048] weight slabs
                wblk = wpool.tile([128, 2048], bf16, tag="w2blk", bufs=2)
                nc.sync.dma_start(wblk[:], w2_t[dc, kg2])
                for i in range(16):
                    kb = 16 * kg2 + i
                    nc.tensor.matmul(
                        ps_z[:], lhsT=wblk[:, i * 128:(i + 1) * 128],
                        rhs=G_sb[kb // 2][:, (kb % 2) * TOK:(kb % 2 + 1) * TOK],
                        start=(kb == 0), stop=(kb == HC - 1))
            zsc = trans.tile([128, TOK], f32, tag="sc_evac", name="zsc",
                             bufs=2)
            if HAS_COL_BIAS:
                nc.vector.tensor_scalar(
                    out=zsc[:], in0=ps_z[:], scalar1=bcol(72 + dc),
                    scalar2=mod_sb[:, 40 + dc:41 + dc],
                    op0=OP.add, op1=OP.mult)
            else:
                nc.vector.tensor_scalar(
                    out=zsc[:], in0=ps_z[:],
                    scalar1=mod_sb[:, 40 + dc:41 + dc], scalar2=None,
                    op0=OP.mult)
            ot = trans.tile([128, TOK], f32, tag="sc_evac", name="ot", bufs=2)
            nc.gpsimd.tensor_tensor(out=ot[:], in0=zsc[:],
                                    in1=x2T[dc][:], op=OP.add)
            out_q[dc % 4].dma_start(out_feat[dc * 128:(dc + 1) * 128, :],
                                    ot[:])

        ctx.close()

    nc.compile()
    return nc


def _pack_bias(bq, bk, bo, b2, bada):
    t = np.zeros((128, 80), np.float32)
    t[:, 0:48] = bada.reshape(48, 128).T
    t[:, 48:56] = bq.reshape(8, 128).T
    t[:, 56:64] = bk.reshape(8, 128).T
    t[:, 64:72] = bo.reshape(8, 128).T
    t[:, 72:80] = b2.reshape(8, 128).T
    return t


def _slab_oc(wT):
    """[D, D] W.T -> [8, 128, 1024]: slab[oc][p][k*128+c] = wT[k*128+p, oc*128+c]"""
    w = wT.reshape(KT, 128, KT, 128)          # [k, p, oc, c]
    return np.ascontiguousarray(w.transpose(2, 1, 0, 3).reshape(KT, 128, D))


WSC = 64.0  # fp8 weight scale (the device folds 1/WSC into the evacuations)


def _fp8(a):
    import ml_dtypes
    return np.clip(a, -240.0, 240.0).astype(ml_dtypes.float8_e4m3)


def _slab_w1(w1T):
    """[D, HID] W1.T -> [8(g4), 128(p), 8(k)*512]: slab[g4][p][k*512+c] =
    w1T[k*128+p, g4*512+c]"""
    w = w1T.reshape(KT, 128, 8, 512)          # [k, p, g4, c]
    return np.ascontiguousarray(w.transpose(2, 1, 0, 3).reshape(KT, 128, 8 * 512))


def _slab_w2(w2T):
    """[HID, D] W2.T -> [8(dc), 2(kg2), 128(p), 16(i)*128]: slab[dc,kg2,p,i*128+c]
    = w2T[(16*kg2+i)*128+p, dc*128+c]"""
    w = w2T.reshape(2, 16, 128, KT, 128)      # [kg2, i, p, dc, c]
    return np.ascontiguousarray(
        w.transpose(3, 0, 2, 1, 4).reshape(KT, 2, 128, 2048))


def _slab_wada(wadaT):
    """[D, 6D] Wada.T -> [48, 128, 1024]: slab[blk][p][k*128+c] =
    wadaT[k*128+p, blk*128+c]"""
    w = wadaT.reshape(KT, 128, 48, 128)       # [k, p, blk, c]
    return np.ascontiguousarray(w.transpose(2, 1, 0, 3).reshape(48, 128, D))


def kernel(x, cond, Wq, bq, Wk, bk, Wv, bv, Wo, bo, W1, b1, W2, b2, Wada, bada):
    import ml_dtypes
    from concourse.bass_utils import run_bass_kernel_spmd

    bf = ml_dtypes.bfloat16
    global HAS_ROW_BIAS, HAS_COL_BIAS
    if "nc" not in _cached:
        HAS_ROW_BIAS = bool(np.any(np.asarray(bv)) or np.any(np.asarray(b1)))
        HAS_COL_BIAS = bool(np.any(np.asarray(bq)) or np.any(np.asarray(bk))
                            or np.any(np.asarray(bo)) or np.any(np.asarray(b2))
                            or np.any(np.asarray(bada)))
        _cached["nc"] = _build()
    nc = _cached["nc"]

    x = np.asarray(x, np.float32)
    cond = np.asarray(cond, np.float32)
    to_bf_T = lambda w: np.ascontiguousarray(
        np.asarray(w, np.float32).T).astype(bf)
    wq_t = _slab_oc(np.asarray(Wq, np.float32).T).astype(bf)
    wk_t = _slab_oc(np.asarray(Wk, np.float32).T).astype(bf)
    wo_t = _slab_oc(np.asarray(Wo, np.float32).T).astype(bf)
    wvT = to_bf_T(Wv)
    w1_t = _slab_w1(np.asarray(W1, np.float32).T).astype(bf)
    w2_t = _slab_w2(np.asarray(W2, np.float32).T).astype(bf)
    wada_b = _slab_wada(np.asarray(Wada, np.float32).T).astype(bf)
    biasc = _pack_bias(np.asarray(bq, np.float32), np.asarray(bk, np.float32),
                       np.asarray(bo, np.float32),
                       np.asarray(b2, np.float32),
                       np.asarray(bada, np.float32))
    bv_row = np.asarray(bv, np.float32).reshape(1, D).astype(bf)

    in_maps = []
    for c in range(N_CORES):
        b, h = c // 2, c % 2
        # own 512 tokens first, then the other half (token-permuted batch)
        xs = np.concatenate([x[b, h * TOK:(h + 1) * TOK, :],
                             x[b, (1 - h) * TOK:(2 - h) * TOK, :]], axis=0)
        in_maps.append({
            "x_feat": np.ascontiguousarray(xs.T).astype(bf),
            "condT": np.ascontiguousarray(cond[b, 0].reshape(8, 128).T),
            "wq_t": wq_t, "wk_t": wk_t, "wo_t": wo_t, "wvT": wvT,
            "w1_t": w1_t, "w2_t": w2_t, "wada_b": wada_b,
            "biasc": biasc, "bv_row": bv_row,
            "b1_row": np.asarray(b1, np.float32).reshape(1, HID).astype(bf),
        })

    _cached["in_maps"] = in_maps
    res = run_bass_kernel_spmd(nc, in_maps, core_ids=list(range(N_CORES)))
    _cached["results"] = res.results
    out = np.empty((B, S, D), np.float32)
    for c in range(N_CORES):
        b, h = c // 2, c % 2
        out[b, h * TOK:(h + 1) * TOK, :] = res.results[c]["out_feat"].T
    return out


# revision 45
# speedup vs baseline: 1.0231x; 1.0210x over previous
"""AdaLN DiT block on 8 Trainium2 NeuronCores — v7, zero collectives.

Sharding: core c owns batch b=c//2 and query-half h=c%2. Host-side the
x tokens are permuted per core so the OWN 512 tokens are always columns
0:512 of x_feat ([D, 1024] feature-major, own|other). Each core computes
LN1 + k/v projections for its FULL batch (1024 tokens) locally. q / Wo /
MLP / residuals are own-half only. No collectives at all.

v7 changes vs v2 (433.5us -> ~395us):
- LN stats matmul pairs col-group-packed: sum in (row 0, bank 0) and
  sumsq in (row 32, bank 1) of one [128, 2*TOK] psum tile. Distinct col
  groups make the pair run concurrently; distinct banks are REQUIRED —
  a start=True bank-clear from one stream lands mid-flight in a
  concurrently-streaming col-tiled matmul sharing the bank and wipes
  its first-touch bits (intermittent negative variance -> NaN).
- ln prep/apply split: per-column a=rstd / b=-mu*rstd rows broadcast
  into one [128, 2*TOK] psum tile; both halves' preps emit before the
  apply chains so the h1 sqrt doesn't head-of-line-block the Scalar
  engine FIFO in front of the projection evacuations.
- Modulates run on the Scalar engine (Identity with AP scale/bias);
  q/k/v PSUM evacuations on Scalar (Copy) — the Vector engine only
  carries the LN mult/add chain in that phase.
- Softmax exp split across engines: head-half 0 exact exp on Scalar,
  head-half 1 via a Schraudolph bf16 bit-trick on Vector (int16 bitcast
  of x*log2(e)*128 + 16248.6; ~2% relative, cancels against the
  denominator computed from the same values).
- Wo is a PSUM-accumulated GEMM after the attention loop (was SBUF f32
  partial accumulation: 64 DVE adds and a ~30us serial tail that let
  HAM re-throttle the PE into fc1). Its LN2 stats matmuls emit after
  the whole GEMM so the PE queue never blocks on the elementwise chain.
- adaLN blocks 0..15 run at the very front (PE warmup during x DMA),
  16..47 interleave into attention hp 0..5.
- Output DMAs spread across 3 queues; x2/sq epilogue split DVE/GpSimd.

PSUM budget (8 banks): tag "big" [128,1024] x2 = 4 banks (stats, ab
broadcasts, scores, fc1), tag "proj" [128,512] x2 = 2 (k/q/v psum,
mod2, Wo, fc2), tag "av" [128,512] x2 = 2 (mod1, AV, psb).
NOTE: matmul start=True clears the WHOLE psum bank; concurrent
(col-tiled) accumulation streams must therefore live in separate banks.
"""

import numpy as np

B, S, D, H, HID = 4, 1024, 1024, 16, 4096
DK = D // H  # 64
N_CORES = 8
TOK = 512    # own tokens per core
SB = 1024    # batch tokens per core (k/v computed locally)
EPS = 1e-6
KT = 8    # 128-row blocks in D
HC = 32   # 128-row blocks in HID

_cached = {}
DEBUG = False
USE_FAST_EXP = True
HAS_ROW_BIAS = False  # bv/b1 nonzero -> adds the rank-1 bias matmuls
HAS_COL_BIAS = False  # bq/bk/bo/b2/bada nonzero -> bias-add epilogues


def _build():
    import contextlib
    import concourse.bass as bass  # noqa: F401
    import concourse.tile as tile
    from concourse import bacc, mybir

    f32 = mybir.dt.float32
    bf16 = mybir.dt.bfloat16
    f8 = mybir.dt.float8e4
    PM = mybir.MatmulPerfMode.DoubleRow
    WSC = 64.0  # host-side fp8 weight scale (descaled in the evacuations)
    ASC = 16.0  # attnT fp8 activation scale (max |attn| <= max |v| ~6)
    AF = mybir.ActivationFunctionType
    OP = mybir.AluOpType

    nc = bacc.Bacc("TRN2", target_bir_lowering=False, debug=False,
                   num_devices=N_CORES)

    # ---- per-core external I/O ----
    x_feat = nc.dram_tensor("x_feat", [D, SB], bf16, kind="ExternalInput")
    condT = nc.dram_tensor("condT", [128, 8], f32, kind="ExternalInput")
    wq_t = nc.dram_tensor("wq_t", [KT, 128, D], bf16, kind="ExternalInput")
    wk_t = nc.dram_tensor("wk_t", [KT, 128, D], bf16, kind="ExternalInput")
    wo_t = nc.dram_tensor("wo_t", [KT, 128, D], bf16, kind="ExternalInput")
    wvT = nc.dram_tensor("wvT", [D, D], bf16, kind="ExternalInput")
    w1_t = nc.dram_tensor("w1_t", [KT, 128, 8 * 512], bf16, kind="ExternalInput")
    w2_t = nc.dram_tensor("w2_t", [KT, 2, 128, 2048], bf16, kind="ExternalInput")
    wada_b = nc.dram_tensor("wada_b", [48, 128, D], bf16, kind="ExternalInput")
    # packed per-partition bias columns (fp32): 0..47 bada, 48..55 bq,
    # 56..63 bk, 64..71 bo, 72..79 b2
    biasc = nc.dram_tensor("biasc", [128, 80], f32, kind="ExternalInput")
    bv_row = nc.dram_tensor("bv_row", [1, D], bf16, kind="ExternalInput")
    b1_row = nc.dram_tensor("b1_row", [1, HID], bf16, kind="ExternalInput")
    out_feat = nc.dram_tensor("out_feat", [D, TOK], f32, kind="ExternalOutput")
    if DEBUG:
        dbg_h1 = nc.dram_tensor("dbg_h1", [4, 128, SB], bf16,
                                kind="ExternalOutput")
        dbg_kT = nc.dram_tensor("dbg_kT", [KT, 128, SB], bf16,
                                kind="ExternalOutput")
        dbg_qT = nc.dram_tensor("dbg_qT", [KT, 128, TOK], bf16,
                                kind="ExternalOutput")
        dbg_v = nc.dram_tensor("dbg_v", [128, 16 * 8 * 65], bf16,
                               kind="ExternalOutput")
        dbg_mod = nc.dram_tensor("dbg_mod", [128, 48], f32,
                                 kind="ExternalOutput")

    with tile.TileContext(nc) as tc:
        ctx = contextlib.ExitStack()
        consts = ctx.enter_context(tc.tile_pool(name="consts", bufs=1))
        persist = ctx.enter_context(tc.tile_pool(name="persist", bufs=1))
        reuse = ctx.enter_context(tc.tile_pool(name="reuse", bufs=1))
        wpool = ctx.enter_context(tc.tile_pool(name="wpool", bufs=3))
        trans = ctx.enter_context(tc.tile_pool(name="trans", bufs=3))
        pT_pool = ctx.enter_context(tc.tile_pool(name="pTp", bufs=2))
        psum = ctx.enter_context(tc.tile_pool(name="psum", bufs=2, space="PSUM"))

        # ---------- constants (cond first — it gates the silu/mod path) ----
        cond_sb = consts.tile([128, 8], f32)
        nc.sync.dma_start(cond_sb[:], condT[:])
        bias_sb = consts.tile([128, 80], f32)
        nc.scalar.dma_start(bias_sb[:], biasc[:])
        if HAS_ROW_BIAS:
            bvr_sb = consts.tile([1, D], bf16)
            nc.scalar.dma_start(bvr_sb[:], bv_row[:])
            b1r_sb = consts.tile([1, HID], bf16)
            nc.scalar.dma_start(b1r_sb[:], b1_row[:])
            ones_tok = consts.tile([1, TOK], bf16)
            nc.vector.memset(ones_tok[:], 1.0)
        ones_col = consts.tile([128, 1], bf16)
        nc.vector.memset(ones_col[:], 1.0)
        eps_sb = consts.tile([1, 1], f32)
        nc.vector.memset(eps_sb[:], EPS)
        ones_m = consts.tile([1, 128], bf16)
        nc.vector.memset(ones_m[:], 1.0)
        ones2 = consts.tile([65, 128], bf16)  # ones rows at partitions 0..64
        nc.vector.memset(ones2[:], 1.0)

        def bcol(i):
            return bias_sb[:, i:i + 1]

        # ---------- adaLN modulation: silu(cond) ----------
        silu_sb = consts.tile([128, 8], bf16)
        nc.scalar.activation(silu_sb[:], cond_sb[:], AF.Silu)

        mod_sb = consts.tile([128, 48], f32)
        mod1p_sb = consts.tile([128, 48], f32)

        def emit_mod_block(ps, col, blk, first, eng=None):
            """One 128-output adaLN block: 256KB DMA + 8 stationary mms."""
            wt = wpool.tile([128, D], bf16, tag="wada", bufs=2, name="wada")
            (eng or nc.sync).dma_start(wt[:], wada_b[blk])
            for k in range(KT):
                nc.tensor.matmul(
                    ps[:, col:col + 1], lhsT=wt[:, k * 128:(k + 1) * 128],
                    rhs=silu_sb[:, k:k + 1],
                    start=(first and k == 0), stop=(k == KT - 1),
                    skip_group_check=True)

        # ---------- phase 1: x DMA + LN1 stats + gama1/beta1 ----------
        xfeat_sb = []
        for d in range(KT):
            xf = persist.tile([128, SB], bf16, tag=f"xfeat{d}", name=f"xf{d}")
            xfeat_sb.append(xf)
        # own-half columns first so stats h0 can start earliest
        for half in range(2):
            for d in range(KT):
                nc.gpsimd.dma_start(
                    xfeat_sb[d][:, half * TOK:(half + 1) * TOK],
                    x_feat[d * 128:(d + 1) * 128, half * TOK:(half + 1) * TOK])

        ps_mod = psum.tile([128, 16], f32, tag="av", bufs=2, name="ps_mod")
        # per-half stats tile: sum in row 0, sumsq in row 32 of one bank;
        # the (sum, sumsq) matmul pair runs concurrently via col-groups.
        # sum in (row 0, bank 0), sumsq in (row 32, bank 1): distinct col
        # groups make the pair concurrent, distinct banks make each stream's
        # start=True bank-clear safe against the other.
        ps_st = [psum.tile([128, 2 * TOK], f32, tag="big", name=f"ps_st{h}")
                 for h in range(2)]

        for half in range(2):
            for d in range(KT):
                xs = xfeat_sb[d][:, half * TOK:(half + 1) * TOK]
                sq = trans.tile([128, TOK], bf16, tag="lnsq", bufs=2,
                                name="lnsq")
                nc.vector.tensor_tensor(out=sq[:], in0=xs, in1=xs, op=OP.mult)
                nc.tensor.matmul(ps_st[half][0:1, 0:TOK], lhsT=ones_col[:],
                                 rhs=xs, start=(d == 0), stop=(d == KT - 1),
                                 skip_group_check=True)
                nc.tensor.matmul(ps_st[half][32:33, TOK:2 * TOK],
                                 lhsT=ones_col[:],
                                 rhs=sq[:], start=(d == 0),
                                 stop=(d == KT - 1),
                                 skip_group_check=True)
                blk = 8 * half + d  # gama1 blocks 0..7 then beta1 8..15
                emit_mod_block(ps_mod, blk, blk, first=(blk == 0))

        if HAS_COL_BIAS:
            nc.vector.tensor_tensor(out=mod_sb[:, 0:16], in0=ps_mod[:],
                                    in1=bias_sb[:, 0:16], op=OP.add)
        else:
            nc.vector.tensor_copy(out=mod_sb[:, 0:16], in_=ps_mod[:])
        nc.vector.tensor_scalar_add(mod1p_sb[:, 0:16], mod_sb[:, 0:16], 1.0)

        # ---------- LN finish helpers (prep: stats->broadcast, apply: per-d) --
        def ln_prep(ps_stat, width):
            """ps_stat rows 0(sum)/32(sumsq) -> psum [128, 2w] = [rstd | -mu*rstd]"""
            stA = trans.tile([1, 6 * width], f32, tag="lnstat", name="lnstat",
                             bufs=1)
            mu_n = stA[:, 0:width]             # -mu
            ex2 = stA[:, width:2 * width]
            var = stA[:, 2 * width:3 * width]
            tmp = stA[:, 3 * width:4 * width]  # mu^2 then std
            rstd = stA[:, 4 * width:5 * width]
            bb = stA[:, 5 * width:6 * width]   # -mu*rstd
            nc.vector.tensor_scalar(out=mu_n, in0=ps_stat[0:1, 0:width],
                                    scalar1=-1.0 / D, scalar2=None,
                                    op0=OP.mult)
            nc.vector.tensor_scalar(out=ex2,
                                    in0=ps_stat[32:33, width:2 * width],
                                    scalar1=1.0 / D, scalar2=None,
                                    op0=OP.mult)
            nc.vector.tensor_tensor(out=tmp, in0=mu_n, in1=mu_n, op=OP.mult)
            nc.vector.tensor_tensor(out=var, in0=ex2, in1=tmp, op=OP.subtract)
            nc.scalar.activation(tmp, var, AF.Sqrt, bias=eps_sb[:], scale=1.0)
            nc.vector.reciprocal_approx_fast(rstd, tmp)
            nc.vector.tensor_tensor(out=bb, in0=mu_n, in1=rstd, op=OP.mult)
            ab_bf = trans.tile([1, 2 * width], bf16, tag="lnstatbf",
                               name="lnstatbf", bufs=1)
            nc.vector.tensor_copy(out=ab_bf[:, 0:width], in_=rstd)
            nc.vector.tensor_copy(out=ab_bf[:, width:2 * width], in_=bb)
            ab = psum.tile([128, 2 * width], f32, tag="big", name="ab_bc")
            nc.tensor.matmul(ab[:, 0:width], lhsT=ones_m[:],
                             rhs=ab_bf[:, 0:width], start=True, stop=True,
                             skip_group_check=True)
            nc.tensor.matmul(ab[:, width:2 * width], lhsT=ones_m[:],
                             rhs=ab_bf[:, width:2 * width], start=True,
                             stop=True, skip_group_check=True)
            return ab

        def ln_apply(ab, width, src_cols, dst, beta_blk, gama_blk,
                     mod_on_act=False, split=False):
            absb = None
            if split:
                absb = trans.tile([128, 2 * width], bf16, tag="absb",
                                  name="absb", bufs=1)
                nc.vector.tensor_copy(out=absb[:], in_=ab[:])
            for d in range(KT):
                on_gp = split and d % 2 == 1
                eng = nc.gpsimd if on_gp else nc.vector
                A = absb if on_gp else ab
                t1 = trans.tile([128, width], bf16, tag="lnt", bufs=4,
                                name="lnt1")
                eng.tensor_tensor(out=t1[:], in0=src_cols(d),
                                  in1=A[:, 0:width], op=OP.mult)
                t2 = trans.tile([128, width], bf16, tag="lnt", bufs=4,
                                name="lnt2")
                eng.tensor_tensor(out=t2[:], in0=t1[:],
                                  in1=A[:, width:2 * width], op=OP.add)
                if mod_on_act:
                    nc.scalar.activation(
                        dst(d), t2[:], AF.Identity,
                        bias=mod_sb[:, gama_blk + d:gama_blk + d + 1],
                        scale=mod1p_sb[:, beta_blk + d:beta_blk + d + 1])
                else:
                    nc.vector.tensor_scalar(
                        out=dst(d), in0=t2[:],
                        scalar1=mod1p_sb[:, beta_blk + d:beta_blk + d + 1],
                        scalar2=mod_sb[:, gama_blk + d:gama_blk + d + 1],
                        op0=OP.mult, op1=OP.add)

        h1T = []
        for d in range(KT):
            h1T.append(reuse.tile([128, SB], bf16, tag=f"rA{d}", bufs=1,
                                  name=f"h1T{d}"))
        ab_h = [ln_prep(ps_st[h], TOK) for h in range(2)]
        for half in range(2):
            c0, c1 = half * TOK, (half + 1) * TOK
            ln_apply(ab_h[half], TOK,
                     lambda d: xfeat_sb[d][:, c0:c1],
                     lambda d: h1T[d][:, c0:c1], 8, 0, mod_on_act=True)

        if DEBUG:
            for d in range(4):
                nc.gpsimd.dma_start(dbg_h1[d], h1T[d][:])
        # ---------- projections ----------
        def evac_proj(dst, ps, bias_i):
            if HAS_COL_BIAS:
                nc.vector.tensor_scalar(out=dst, in0=ps[:],
                                        scalar1=bcol(bias_i), scalar2=None,
                                        op0=OP.add)
            else:
                nc.scalar.activation(dst, ps[:], AF.Copy)

        kT = []
        for oc in range(KT):
            kT.append(reuse.tile([128, SB], bf16, tag=f"rB{oc}", bufs=1,
                                 name=f"kT{oc}"))
        # half-outer so the own-half k projection isn't gated on the
        # other half's modulate; wk slabs stay resident across both halves
        wk_sb = []
        for oc in range(KT):
            wblk = wpool.tile([128, D], bf16, tag="wblk", bufs=8)
            nc.sync.dma_start(wblk[:], wk_t[oc])
            wk_sb.append(wblk)
        for half in range(2):
            c0, c1 = half * TOK, (half + 1) * TOK
            for oc in range(KT):
                ps = psum.tile([128, TOK], f32, tag="proj")
                for k in range(KT):
                    nc.tensor.matmul(
                        ps[:], lhsT=wk_sb[oc][:, k * 128:(k + 1) * 128],
                        rhs=h1T[k][:, c0:c1], start=(k == 0),
                        stop=(k == KT - 1))
                evac_proj(kT[oc][:, c0:c1], ps, 56 + oc)

        if DEBUG:
            for oc in range(KT):
                nc.gpsimd.dma_start(dbg_kT[oc], kT[oc][:])
        # q: own half only
        qT = []
        for oc in range(KT):
            wblk = wpool.tile([128, D], bf16, tag="wblk", bufs=8)
            nc.sync.dma_start(wblk[:], wq_t[oc])
            ps = psum.tile([128, TOK], f32, tag="proj")
            for k in range(KT):
                nc.tensor.matmul(
                    ps[:], lhsT=wblk[:, k * 128:(k + 1) * 128],
                    rhs=h1T[k][:, 0:TOK], start=(k == 0), stop=(k == KT - 1))
            qt = persist.tile([128, TOK], bf16, tag=f"qT{oc}", name=f"qT{oc}")
            evac_proj(qt[:], ps, 48 + oc)
            qT.append(qt)

        if DEBUG:
            for oc in range(KT):
                nc.gpsimd.dma_start(dbg_qT[oc], qT[oc][:])
        # v: token-major over the full batch, evacuated straight into the
        # padded per-head layout [128, head, kc, 65] (col 64 = ones).
        v_h8 = persist.tile([128, 16, 8, 65], bf16, tag="v_h8", name="v_h8")
        nc.vector.memset(v_h8[:, :, :, 64:65], 1.0)
        for fh in range(2):
            wv_tiles = []
            for k in range(KT):
                wblk = wpool.tile([128, TOK], bf16, tag="wvblk", name="wvblk",
                                  bufs=8)
                nc.sync.dma_start(
                    wblk[:], wvT[k * 128:(k + 1) * 128,
                                 fh * 512:(fh + 1) * 512])
                wv_tiles.append(wblk)
            for tb in range(8):
                ps = psum.tile([128, TOK], f32, tag="proj")
                for k in range(KT):
                    nc.tensor.matmul(
                        ps[:], lhsT=h1T[k][:, tb * 128:(tb + 1) * 128],
                        rhs=wv_tiles[k][:],
                        start=(k == 0),
                        stop=(k == KT - 1 and not HAS_ROW_BIAS),
                        skip_group_check=True)
                if HAS_ROW_BIAS:
                    nc.tensor.matmul(
                        ps[:], lhsT=ones_m[:],
                        rhs=bvr_sb[:, fh * 512:(fh + 1) * 512],
                        start=False, stop=True)
                # strided evac: [128, 512] -> heads fh*8..fh*8+7, kc=tb
                nc.scalar.activation(
                    v_h8[:, fh * 8:(fh + 1) * 8, tb, 0:64],
                    ps[:].rearrange("p (h c) -> p h c", h=8), AF.Copy)

        if DEBUG:
            nc.gpsimd.dma_start(dbg_v[:], v_h8[:].rearrange("p a b c -> p (a b c)"))
        # ---------- attention (+ interleaved adaLN blocks 32..47) ----------
        ps_mod2 = psum.tile([128, 32], f32, tag="proj", name="ps_mod2")
        MOD2_PER_HP = [4, 4, 4, 4, 4, 4, 4, 4]
        nmod2 = 0
        attnT = []
        for hp in range(KT):
            attnT.append(persist.tile([128, TOK], bf16, tag=f"attnT{hp}",
                                      name=f"attnT{hp}"))
        SCALE = 1.0 / 8.0

        # Wo weight slabs prefetched; the GEMM itself runs after the loop.
        wo_sb = []
        for dc in range(KT):
            wblk = wpool.tile([128, D], bf16, tag="wblk", bufs=8)
            nc.sync.dma_start(wblk[:], wo_t[dc])
            wo_sb.append(wblk)

        # Schraudolph fast-exp constants for the DVE path (bf16 bit trick):
        # bits16 = x*SCALE*log2(e)*128 + (127*128 - 7.41); rel err ~2% which
        # cancels between numerator and denominator of the softmax.
        EXP_MUL = SCALE * 1.4426950408889634 * 128.0
        EXP_ADD = 16256.0 - 7.41

        for hp in range(KT):
            pT_g = {}
            for g in range(4):
                ps_AB = [psum.tile([128, 2 * TOK], f32, tag="big",
                                   name=f"ps_s{hh}") for hh in range(2)]
                for i in range(2):
                    kc = 2 * g + i
                    for hh in range(2):
                        nc.tensor.matmul(
                            ps_AB[hh][:, i * TOK:(i + 1) * TOK],
                            lhsT=kT[hp][hh * 64:(hh + 1) * 64,
                                        kc * 128:(kc + 1) * 128],
                            rhs=qT[hp][hh * 64:(hh + 1) * 64, :],
                            start=True, stop=True)
                # hh=0 exact exp on ACT; hh=1 fast-exp on DVE
                pt = pT_pool.tile([128, 2 * TOK], bf16, tag="pT",
                                  name="pTg", bufs=5)
                nc.scalar.activation(out=pt[:], in_=ps_AB[0][:],
                                     func=AF.Exp, scale=SCALE)
                pT_g[(0, g)] = pt
                pti = pT_pool.tile([128, 2 * TOK], bf16, tag="pTi",
                                   name="pTi", bufs=5)
                if USE_FAST_EXP:
                    nc.vector.tensor_scalar(
                        out=pti[:].bitcast(mybir.dt.int16), in0=ps_AB[1][:],
                        scalar1=EXP_MUL, scalar2=EXP_ADD,
                        op0=OP.mult, op1=OP.add)
                else:
                    nc.scalar.activation(out=pti[:], in_=ps_AB[1][:],
                                         func=AF.Exp, scale=SCALE)
                pT_g[(1, g)] = pti
            ps_avs = []
            for hh in range(2):
                h = 2 * hp + hh
                ps_av = psum.tile([128, TOK], f32, tag="av")
                for kc in range(8):
                    nc.tensor.matmul(
                        ps_av[0:65, :], lhsT=v_h8[:, h, kc, :],
                        rhs=pT_g[(hh, kc // 2)][:, (kc % 2) * TOK:
                                                (kc % 2 + 1) * TOK],
                        start=(kc == 0), stop=(kc == 7))
                ps_avs.append(ps_av)
                # interleave adaLN blocks 16..47
                for _ in range(MOD2_PER_HP[hp] // 2):
                    if nmod2 < 32:
                        emit_mod_block(ps_mod2, nmod2, 16 + nmod2,
                                       first=(nmod2 == 0), eng=nc.gpsimd)
                        nmod2 += 1
            # per-hp softmax normalization: denominators live in row 64 of
            # each ps_av; stage both heads' reciprocals at partitions 0/64
            # (matmul rhs base must be 0/32/64)
            dn2 = trans.tile([65, 2 * TOK], f32, tag="dn", bufs=1, name="dn2")
            for hh in range(2):
                nc.vector.tensor_copy(out=dn2[64 * hh:64 * hh + 1, 0:TOK],
                                      in_=ps_avs[hh][64:65, :])
            nc.vector.reciprocal_approx_fast(dn2[:, TOK:2 * TOK],
                                             dn2[:, 0:TOK])
            rd2 = trans.tile([65, TOK], bf16, tag="rd", bufs=1, name="rd2")
            for hh in range(2):
                nc.vector.tensor_copy(
                    out=rd2[64 * hh:64 * hh + 1, :],
                    in_=dn2[64 * hh:64 * hh + 1, TOK:2 * TOK])
            for hh in range(2):
                psb = psum.tile([128, TOK], f32, tag="av", name="psb")
                nc.tensor.matmul(psb[0:64, :],
                                 lhsT=ones2[64 * hh:64 * hh + 1, 0:64],
                                 rhs=rd2[64 * hh:64 * hh + 1, :],
                                 start=True, stop=True)
                nc.vector.tensor_copy(out=attnT[hp][hh * 64:(hh + 1) * 64, :],
                                      in_=ps_avs[hh][0:64, :])
                nc.vector.tensor_tensor(
                    out=attnT[hp][hh * 64:(hh + 1) * 64, :],
                    in0=attnT[hp][hh * 64:(hh + 1) * 64, :],
                    in1=psb[0:64, :], op=OP.mult)

        # evacuate adaLN blocks 16..47 (alpha1, gama2, beta2, alpha2)
        if HAS_COL_BIAS:
            nc.vector.tensor_tensor(out=mod_sb[:, 16:48], in0=ps_mod2[:],
                                    in1=bias_sb[:, 16:48], op=OP.add)
        else:
            nc.vector.tensor_copy(out=mod_sb[:, 16:48], in_=ps_mod2[:])
        nc.vector.tensor_scalar_add(mod1p_sb[:, 16:48], mod_sb[:, 16:48], 1.0)

        if DEBUG:
            nc.gpsimd.dma_start(dbg_mod[:], mod_sb[:])
        # ---------- Wo GEMM + epilogue fused with LN2 stats ----------
        # All 64 Wo matmuls first (dense PE stream); the per-dc evac chains
        # (DVE/GpSimd) trail behind; the stats matmuls go after so they
        # don't block the PE queue on the elementwise chain.
        x2T = []
        sq2 = []
        ps_st2 = psum.tile([128, 2 * TOK], f32, tag="big", name="ps_st2")
        for dc in range(KT):
            ps_w = psum.tile([128, TOK], f32, tag="proj", name="ps_wo")
            for hp in range(KT):
                nc.tensor.matmul(ps_w[:],
                                 lhsT=wo_sb[dc][:, hp * 128:(hp + 1) * 128],
                                 rhs=attnT[hp][:], start=(hp == 0),
                                 stop=(hp == KT - 1))
            ysc = trans.tile([128, TOK], f32, tag="sc_evac", name="ysc",
                             bufs=2)
            if HAS_COL_BIAS:
                nc.vector.tensor_scalar(
                    out=ysc[:], in0=ps_w[:], scalar1=bcol(64 + dc),
                    scalar2=mod_sb[:, 16 + dc:17 + dc], op0=OP.add,
                    op1=OP.mult)
            else:
                nc.vector.tensor_scalar(
                    out=ysc[:], in0=ps_w[:],
                    scalar1=mod_sb[:, 16 + dc:17 + dc], scalar2=None,
                    op0=OP.mult)
            x2t = persist.tile([128, TOK], bf16, tag=f"x2T{dc}",
                               name=f"x2T{dc}")
            nc.vector.tensor_tensor(out=x2t[:], in0=ysc[:],
                                    in1=xfeat_sb[dc][:, 0:TOK], op=OP.add)
            x2T.append(x2t)
            sq = trans.tile([128, TOK], bf16, tag="sq2", bufs=6, name="sq2")
            nc.gpsimd.tensor_tensor(out=sq[:], in0=x2t[:], in1=x2t[:],
                                    op=OP.mult)
            sq2.append(sq)
        for dc in range(KT):
            nc.tensor.matmul(ps_st2[0:1, 0:TOK], lhsT=ones_col[:],
                             rhs=x2T[dc][:], start=(dc == 0),
                             stop=(dc == KT - 1), skip_group_check=True)
            nc.tensor.matmul(ps_st2[32:33, TOK:2 * TOK], lhsT=ones_col[:],
                             rhs=sq2[dc][:], start=(dc == 0),
                             stop=(dc == KT - 1), skip_group_check=True)

        h2T = []
        for d in range(KT):
            h2T.append(persist.tile([128, TOK], bf16, tag=f"h2T{d}",
                                    name=f"h2T{d}"))
        ab2 = ln_prep(ps_st2, TOK)
        ps_warm = psum.tile([128, TOK], f32, tag="av", name="ps_warm")
        for i in range(14):
            nc.tensor.matmul(ps_warm[0:1, :], lhsT=ones_col[:],
                             rhs=xfeat_sb[i % 8][:, 0:TOK],
                             start=True, stop=True, skip_group_check=True)
        ln_apply(ab2, TOK, lambda d: x2T[d][:], lambda d: h2T[d][:],
                 32, 24, mod_on_act=True, split=True)

        # ---------- MLP (bf16, token-local) ----------
        G_sb = []
        for g4 in range(8):  # groups of 4 HID blocks
            w1q = wpool.tile([128, 8 * 512], bf16, tag="w1q", bufs=2)
            nc.sync.dma_start(w1q[:], w1_t[g4])
            for jp in range(2):
                ps_g = psum.tile([128, 2 * TOK], f32, tag="big")
                for j2 in range(2):
                    hc = 4 * g4 + 2 * jp + j2
                    korder = [0, 2, 4, 6, 1, 3, 5, 7]
                    for ki, k in enumerate(korder):
                        nc.tensor.matmul(
                            ps_g[:, j2 * TOK:(j2 + 1) * TOK],
                            lhsT=w1q[:, k * 512 + (2 * jp + j2) * 128:
                                     k * 512 + (2 * jp + j2 + 1) * 128],
                            rhs=h2T[k][:],
                            start=(ki == 0), stop=False,
                            skip_group_check=True)
                    if HAS_ROW_BIAS:
                        nc.tensor.matmul(
                            ps_g[:, j2 * TOK:(j2 + 1) * TOK],
                            lhsT=b1r_sb[:, hc * 128:(hc + 1) * 128],
                            rhs=ones_tok[:], start=False, stop=True,
                            skip_group_check=True)
                # reuse the h1T ring (idx 0..7) then the kT ring (8..15);
                # G holds the (2gi | 2gi+1) HID-chunk pair in fp8.
                gi = 2 * g4 + jp
                if gi < 8:
                    gt = reuse.tile([128, SB], bf16, tag=f"rA{gi}", bufs=1,
                                    name=f"G{gi}")
                else:
                    gt = reuse.tile([128, SB], bf16, tag=f"rB{gi - 8}", bufs=1,
                                    name=f"G{gi}")
                nc.scalar.activation(out=gt[:], in_=ps_g[:], func=AF.Gelu)
                G_sb.append(gt)

        out_q = [nc.sync, nc.scalar, nc.gpsimd, nc.scalar]
        for dc in range(KT):
            ps_z = psum.tile([128, TOK], f32, tag="proj")
            for kg2 in range(2):  # two [128, 2048] weight slabs
                wblk = wpool.tile([128, 2048], bf16, tag="w2blk", bufs=2)
                nc.sync.dma_start(wblk[:], w2_t[dc, kg2])
                for i in range(16):
                    kb = 16 * kg2 + i
                    nc.tensor.matmul(
                        ps_z[:], lhsT=wblk[:, i * 128:(i + 1) * 128],
                        rhs=G_sb[kb // 2][:, (kb % 2) * TOK:(kb % 2 + 1) * TOK],
                        start=(kb == 0), stop=(kb == HC - 1))
            zsc = trans.tile([128, TOK], f32, tag="sc_evac", name="zsc",
                             bufs=2)
            if HAS_COL_BIAS:
                nc.vector.tensor_scalar(
                    out=zsc[:], in0=ps_z[:], scalar1=bcol(72 + dc),
                    scalar2=mod_sb[:, 40 + dc:41 + dc],
                    op0=OP.add, op1=OP.mult)
            else:
                nc.vector.tensor_scalar(
                    out=zsc[:], in0=ps_z[:],
                    scalar1=mod_sb[:, 40 + dc:41 + dc], scalar2=None,
                    op0=OP.mult)
            ot = trans.tile([128, TOK], f32, tag="sc_evac", name="ot", bufs=2)
            nc.gpsimd.tensor_tensor(out=ot[:], in0=zsc[:],
                                    in1=x2T[dc][:], op=OP.add)
            out_q[dc % 4].dma_start(out_feat[dc * 128:(dc + 1) * 128, :],
                                    ot[:])

        ctx.close()

    nc.compile()
    return nc


def _pack_bias(bq, bk, bo, b2, bada):
    t = np.zeros((128, 80), np.float32)
    t[:, 0:48] = bada.reshape(48, 128).T
    t[:, 48:56] = bq.reshape(8, 128).T
    t[:, 56:64] = bk.reshape(8, 128).T
    t[:, 64:72] = bo.reshape(8, 128).T
    t[:, 72:80] = b2.reshape(8, 128).T
    return t


def _slab_oc(wT):
    """[D, D] W.T -> [8, 128, 1024]: slab[oc][p][k*128+c] = wT[k*128+p, oc*128+c]"""
    w = wT.reshape(KT, 128, KT, 128)          # [k, p, oc, c]
    return np.ascontiguousarray(w.transpose(2, 1, 0, 3).reshape(KT, 128, D))


WSC = 64.0  # fp8 weight scale (the device folds 1/WSC into the evacuations)


def _fp8(a):
    import ml_dtypes
    return np.clip(a, -240.0, 240.0).astype(ml_dtypes.float8_e4m3)


def _slab_w1(w1T):
    """[D, HID] W1.T -> [8(g4), 128(p), 8(k)*512]: slab[g4][p][k*512+c] =
    w1T[k*128+p, g4*512+c]"""
    w = w1T.reshape(KT, 128, 8, 512)          # [k, p, g4, c]
    return np.ascontiguousarray(w.transpose(2, 1, 0, 3).reshape(KT, 128, 8 * 512))


def _slab_w2(w2T):
    """[HID, D] W2.T -> [8(dc), 2(kg2), 128(p), 16(i)*128]: slab[dc,kg2,p,i*128+c]
    = w2T[(16*kg2+i)*128+p, dc*128+c]"""
    w = w2T.reshape(2, 16, 128, KT, 128)      # [kg2, i, p, dc, c]
    return np.ascontiguousarray(
        w.transpose(3, 0, 2, 1, 4).reshape(KT, 2, 128, 2048))


def _slab_wada(wadaT):
    """[D, 6D] Wada.T -> [48, 128, 1024]: slab[blk][p][k*128+c] =
    wadaT[k*128+p, blk*128+c]"""
    w = wadaT.reshape(KT, 128, 48, 128)       # [k, p, blk, c]
    return np.ascontiguousarray(w.transpose(2, 1, 0, 3).reshape(48, 128, D))


def kernel(x, cond, Wq, bq, Wk, bk, Wv, bv, Wo, bo, W1, b1, W2, b2, Wada, bada):
    import ml_dtypes
    from concourse.bass_utils import run_bass_kernel_spmd

    bf = ml_dtypes.bfloat16
    global HAS_ROW_BIAS, HAS_COL_BIAS
    if "nc" not in _cached:
        HAS_ROW_BIAS = bool(np.any(np.asarray(bv)) or np.any(np.asarray(b1)))
        HAS_COL_BIAS = bool(np.any(np.asarray(bq)) or np.any(np.asarray(bk))
                            or np.any(np.asarray(bo)) or np.any(np.asarray(b2))
                            or np.any(np.asarray(bada)))
        _cached["nc"] = _build()
    nc = _cached["nc"]

    x = np.asarray(x, np.float32)
    cond = np.asarray(cond, np.float32)
    to_bf_T = lambda w: np.ascontiguousarray(
        np.asarray(w, np.float32).T).astype(bf)
    wq_t = _slab_oc(np.asarray(Wq, np.float32).T).astype(bf)
    wk_t = _slab_oc(np.asarray(Wk, np.float32).T).astype(bf)
    wo_t = _slab_oc(np.asarray(Wo, np.float32).T).astype(bf)
    wvT = to_bf_T(Wv)
    w1_t = _slab_w1(np.asarray(W1, np.float32).T).astype(bf)
    w2_t = _slab_w2(np.asarray(W2, np.float32).T).astype(bf)
    wada_b = _slab_wada(np.asarray(Wada, np.float32).T).astype(bf)
    biasc = _pack_bias(np.asarray(bq, np.float32), np.asarray(bk, np.float32),
                       np.asarray(bo, np.float32),
                       np.asarray(b2, np.float32),
                       np.asarray(bada, np.float32))
    bv_row = np.asarray(bv, np.float32).reshape(1, D).astype(bf)

    in_maps = []
    for c in range(N_CORES):
        b, h = c // 2, c % 2
        # own 512 tokens first, then the other half (token-permuted batch)
        xs = np.concatenate([x[b, h * TOK:(h + 1) * TOK, :],
                             x[b, (1 - h) * TOK:(2 - h) * TOK, :]], axis=0)
        in_maps.append({
            "x_feat": np.ascontiguousarray(xs.T).astype(bf),
            "condT": np.ascontiguousarray(cond[b, 0].reshape(8, 128).T),
            "wq_t": wq_t, "wk_t": wk_t, "wo_t": wo_t, "wvT": wvT,
            "w1_t": w1_t, "w2_t": w2_t, "wada_b": wada_b,
            "biasc": biasc, "bv_row": bv_row,
            "b1_row": np.asarray(b1, np.float32).reshape(1, HID).astype(bf),
        })

    _cached["in_maps"] = in_maps
    res = run_bass_kernel_spmd(nc, in_maps, core_ids=list(range(N_CORES)))
    _cached["results"] = res.results
    out = np.empty((B, S, D), np.float32)
    for c in range(N_CORES):
        b, h = c // 2, c % 2
        out[b, h * TOK:(h + 1) * TOK, :] = res.results[c]["out_feat"].T
    return out


# revision 47
# speedup vs baseline: 1.0263x; 1.0031x over previous
"""AdaLN DiT block on 8 Trainium2 NeuronCores — v12, zero collectives.

Sharding: core c owns batch b=c//2 and query-half h=c%2. Host-side the
x tokens are permuted per core so the OWN 512 tokens are always columns
0:512 of x_feat ([D, 1024] feature-major, own|other). Each core computes
LN1 + k/v projections for its FULL batch (1024 tokens) locally. q / Wo /
MLP / residuals are own-half only. No collectives at all.

v7-v12 changes vs v2 (433.5us -> ~391us):
- LN stats matmul pairs col-group-packed: sum in (row 0, bank 0) and
  sumsq in (row 32, bank 1) of one [128, 2*TOK] psum tile. Distinct col
  groups make the pair run concurrently; distinct banks are REQUIRED —
  a start=True bank-clear from one stream lands mid-flight in a
  concurrently-streaming col-tiled matmul sharing the bank and wipes
  its first-touch bits (intermittent negative variance -> NaN).
- ln prep/apply split: per-column a=rstd / b=-mu*rstd rows broadcast
  into one [128, 2*TOK] psum tile; both halves' preps emit before the
  apply chains so the h1 sqrt doesn't head-of-line-block the Scalar
  engine FIFO in front of the projection evacuations.
- Modulates run on the Scalar engine (Identity with AP scale/bias);
  q/k/v PSUM evacuations on Scalar (Copy) — the Vector engine only
  carries the LN mult/add chain in that phase.
- Softmax exp split across engines: head-half 0 exact exp on Scalar,
  head-half 1 via a Schraudolph bf16 bit-trick on Vector (int16 bitcast
  of x*log2(e)*128 + 16248.6; ~2% relative, cancels against the
  denominator computed from the same values).
- Wo is a PSUM-accumulated GEMM after the attention loop (was SBUF f32
  partial accumulation: 64 DVE adds and a ~30us serial tail that let
  HAM re-throttle the PE into fc1). Its LN2 stats matmuls emit after
  the whole GEMM so the PE queue never blocks on the elementwise chain.
- adaLN blocks 0..15 run at the very front (PE warmup during x DMA),
  16..47 interleave evenly into attention hp 0..7 as PE filler.
- Output DMAs spread across 3 queues; x2/sq epilogue split DVE/GpSimd.

PSUM budget (8 banks): tag "big" [128,1024] x2 = 4 banks (stats, ab
broadcasts, scores, fc1), tag "proj" [128,512] x2 = 2 (k/q/v psum,
mod2, Wo, fc2), tag "av" [128,512] x2 = 2 (mod1, AV, psb).
NOTE: matmul start=True clears the WHOLE psum bank; concurrent
(col-tiled) accumulation streams must therefore live in separate banks.
"""

import numpy as np

B, S, D, H, HID = 4, 1024, 1024, 16, 4096
DK = D // H  # 64
N_CORES = 8
TOK = 512    # own tokens per core
SB = 1024    # batch tokens per core (k/v computed locally)
EPS = 1e-6
KT = 8    # 128-row blocks in D
HC = 32   # 128-row blocks in HID

_cached = {}
DEBUG = False
USE_FAST_EXP = True
HAS_ROW_BIAS = False  # bv/b1 nonzero -> adds the rank-1 bias matmuls
HAS_COL_BIAS = False  # bq/bk/bo/b2/bada nonzero -> bias-add epilogues


def _build():
    import contextlib
    import concourse.bass as bass  # noqa: F401
    import concourse.tile as tile
    from concourse import bacc, mybir

    f32 = mybir.dt.float32
    bf16 = mybir.dt.bfloat16
    f8 = mybir.dt.float8e4
    PM = mybir.MatmulPerfMode.DoubleRow
    WSC = 64.0  # host-side fp8 weight scale (descaled in the evacuations)
    ASC = 16.0  # attnT fp8 activation scale (max |attn| <= max |v| ~6)
    AF = mybir.ActivationFunctionType
    OP = mybir.AluOpType

    nc = bacc.Bacc("TRN2", target_bir_lowering=False, debug=False,
                   num_devices=N_CORES)

    # ---- per-core external I/O ----
    x_feat = nc.dram_tensor("x_feat", [D, SB], bf16, kind="ExternalInput")
    condT = nc.dram_tensor("condT", [128, 8], f32, kind="ExternalInput")
    wq_t = nc.dram_tensor("wq_t", [KT, 128, D], bf16, kind="ExternalInput")
    wk_t = nc.dram_tensor("wk_t", [KT, 128, D], bf16, kind="ExternalInput")
    wo_t = nc.dram_tensor("wo_t", [KT, 128, D], bf16, kind="ExternalInput")
    wvT = nc.dram_tensor("wvT", [D, D], bf16, kind="ExternalInput")
    w1_t = nc.dram_tensor("w1_t", [KT, 128, 8 * 512], bf16, kind="ExternalInput")
    w2_t = nc.dram_tensor("w2_t", [KT, 2, 128, 2048], bf16, kind="ExternalInput")
    wada_b = nc.dram_tensor("wada_b", [48, 128, D], bf16, kind="ExternalInput")
    # packed per-partition bias columns (fp32): 0..47 bada, 48..55 bq,
    # 56..63 bk, 64..71 bo, 72..79 b2
    biasc = nc.dram_tensor("biasc", [128, 80], f32, kind="ExternalInput")
    bv_row = nc.dram_tensor("bv_row", [1, D], bf16, kind="ExternalInput")
    b1_row = nc.dram_tensor("b1_row", [1, HID], bf16, kind="ExternalInput")
    out_feat = nc.dram_tensor("out_feat", [D, TOK], f32, kind="ExternalOutput")
    if DEBUG:
        dbg_h1 = nc.dram_tensor("dbg_h1", [4, 128, SB], bf16,
                                kind="ExternalOutput")
        dbg_kT = nc.dram_tensor("dbg_kT", [KT, 128, SB], bf16,
                                kind="ExternalOutput")
        dbg_qT = nc.dram_tensor("dbg_qT", [KT, 128, TOK], bf16,
                                kind="ExternalOutput")
        dbg_v = nc.dram_tensor("dbg_v", [128, 16 * 8 * 65], bf16,
                               kind="ExternalOutput")
        dbg_mod = nc.dram_tensor("dbg_mod", [128, 48], f32,
                                 kind="ExternalOutput")

    with tile.TileContext(nc) as tc:
        ctx = contextlib.ExitStack()
        consts = ctx.enter_context(tc.tile_pool(name="consts", bufs=1))
        persist = ctx.enter_context(tc.tile_pool(name="persist", bufs=1))
        reuse = ctx.enter_context(tc.tile_pool(name="reuse", bufs=1))
        wpool = ctx.enter_context(tc.tile_pool(name="wpool", bufs=3))
        trans = ctx.enter_context(tc.tile_pool(name="trans", bufs=3))
        pT_pool = ctx.enter_context(tc.tile_pool(name="pTp", bufs=2))
        psum = ctx.enter_context(tc.tile_pool(name="psum", bufs=2, space="PSUM"))

        # ---------- constants (cond first — it gates the silu/mod path) ----
        cond_sb = consts.tile([128, 8], f32)
        nc.sync.dma_start(cond_sb[:], condT[:])
        bias_sb = consts.tile([128, 80], f32)
        nc.scalar.dma_start(bias_sb[:], biasc[:])
        if HAS_ROW_BIAS:
            bvr_sb = consts.tile([1, D], bf16)
            nc.scalar.dma_start(bvr_sb[:], bv_row[:])
            b1r_sb = consts.tile([1, HID], bf16)
            nc.scalar.dma_start(b1r_sb[:], b1_row[:])
            ones_tok = consts.tile([1, TOK], bf16)
            nc.vector.memset(ones_tok[:], 1.0)
        ones_col = consts.tile([128, 1], bf16)
        nc.vector.memset(ones_col[:], 1.0)
        eps_sb = consts.tile([1, 1], f32)
        nc.vector.memset(eps_sb[:], EPS)
        ones_m = consts.tile([1, 128], bf16)
        nc.vector.memset(ones_m[:], 1.0)
        ones2 = consts.tile([65, 128], bf16)  # ones rows at partitions 0..64
        nc.vector.memset(ones2[:], 1.0)

        def bcol(i):
            return bias_sb[:, i:i + 1]

        # ---------- adaLN modulation: silu(cond) ----------
        silu_sb = consts.tile([128, 8], bf16)
        nc.scalar.activation(silu_sb[:], cond_sb[:], AF.Silu)

        mod_sb = consts.tile([128, 48], f32)
        mod1p_sb = consts.tile([128, 48], f32)

        def emit_mod_block(ps, col, blk, first, eng=None):
            """One 128-output adaLN block: 256KB DMA + 8 stationary mms."""
            wt = wpool.tile([128, D], bf16, tag="wada", bufs=2, name="wada")
            (eng or nc.sync).dma_start(wt[:], wada_b[blk])
            for k in range(KT):
                nc.tensor.matmul(
                    ps[:, col:col + 1], lhsT=wt[:, k * 128:(k + 1) * 128],
                    rhs=silu_sb[:, k:k + 1],
                    start=(first and k == 0), stop=(k == KT - 1),
                    skip_group_check=True)

        # ---------- phase 1: x DMA + LN1 stats + gama1/beta1 ----------
        xfeat_sb = []
        for d in range(KT):
            xf = persist.tile([128, SB], bf16, tag=f"xfeat{d}", name=f"xf{d}")
            xfeat_sb.append(xf)
        # own-half columns on gpsimd; other half on the scalar queue so
        # both halves land in parallel (~8us earlier h1 -> earlier LN1-h1,
        # k projection). The scalar queue only carries biasc this early.
        for half in range(2):
            for d in range(KT):
                (nc.gpsimd if half == 0 else nc.scalar).dma_start(
                    xfeat_sb[d][:, half * TOK:(half + 1) * TOK],
                    x_feat[d * 128:(d + 1) * 128, half * TOK:(half + 1) * TOK])

        ps_mod = psum.tile([128, 16], f32, tag="av", bufs=2, name="ps_mod")
        # per-half stats tile: sum in row 0, sumsq in row 32 of one bank;
        # the (sum, sumsq) matmul pair runs concurrently via col-groups.
        # sum in (row 0, bank 0), sumsq in (row 32, bank 1): distinct col
        # groups make the pair concurrent, distinct banks make each stream's
        # start=True bank-clear safe against the other.
        ps_st = [psum.tile([128, 2 * TOK], f32, tag="big", name=f"ps_st{h}")
                 for h in range(2)]

        for half in range(2):
            for d in range(KT):
                xs = xfeat_sb[d][:, half * TOK:(half + 1) * TOK]
                sq = trans.tile([128, TOK], bf16, tag="lnsq", bufs=2,
                                name="lnsq")
                nc.vector.tensor_tensor(out=sq[:], in0=xs, in1=xs, op=OP.mult)
                nc.tensor.matmul(ps_st[half][0:1, 0:TOK], lhsT=ones_col[:],
                                 rhs=xs, start=(d == 0), stop=(d == KT - 1),
                                 skip_group_check=True)
                nc.tensor.matmul(ps_st[half][32:33, TOK:2 * TOK],
                                 lhsT=ones_col[:],
                                 rhs=sq[:], start=(d == 0),
                                 stop=(d == KT - 1),
                                 skip_group_check=True)
                blk = 8 * half + d  # gama1 blocks 0..7 then beta1 8..15
                emit_mod_block(ps_mod, blk, blk, first=(blk == 0))

        if HAS_COL_BIAS:
            nc.vector.tensor_tensor(out=mod_sb[:, 0:16], in0=ps_mod[:],
                                    in1=bias_sb[:, 0:16], op=OP.add)
        else:
            nc.vector.tensor_copy(out=mod_sb[:, 0:16], in_=ps_mod[:])
        nc.vector.tensor_scalar_add(mod1p_sb[:, 0:16], mod_sb[:, 0:16], 1.0)

        # ---------- LN finish helpers (prep: stats->broadcast, apply: per-d) --
        def ln_prep(ps_stat, width):
            """ps_stat rows 0(sum)/32(sumsq) -> psum [128, 2w] = [rstd | -mu*rstd]"""
            stA = trans.tile([1, 6 * width], f32, tag="lnstat", name="lnstat",
                             bufs=1)
            mu_n = stA[:, 0:width]             # -mu
            ex2 = stA[:, width:2 * width]
            var = stA[:, 2 * width:3 * width]
            tmp = stA[:, 3 * width:4 * width]  # mu^2 then std
            rstd = stA[:, 4 * width:5 * width]
            bb = stA[:, 5 * width:6 * width]   # -mu*rstd
            nc.vector.tensor_scalar(out=mu_n, in0=ps_stat[0:1, 0:width],
                                    scalar1=-1.0 / D, scalar2=None,
                                    op0=OP.mult)
            nc.vector.tensor_scalar(out=ex2,
                                    in0=ps_stat[32:33, width:2 * width],
                                    scalar1=1.0 / D, scalar2=None,
                                    op0=OP.mult)
            nc.vector.tensor_tensor(out=tmp, in0=mu_n, in1=mu_n, op=OP.mult)
            nc.vector.tensor_tensor(out=var, in0=ex2, in1=tmp, op=OP.subtract)
            nc.scalar.activation(tmp, var, AF.Sqrt, bias=eps_sb[:], scale=1.0)
            nc.vector.reciprocal_approx_fast(rstd, tmp)
            nc.vector.tensor_tensor(out=bb, in0=mu_n, in1=rstd, op=OP.mult)
            ab_bf = trans.tile([1, 2 * width], bf16, tag="lnstatbf",
                               name="lnstatbf", bufs=1)
            nc.vector.tensor_copy(out=ab_bf[:, 0:width], in_=rstd)
            nc.vector.tensor_copy(out=ab_bf[:, width:2 * width], in_=bb)
            ab = psum.tile([128, 2 * width], f32, tag="big", name="ab_bc")
            nc.tensor.matmul(ab[:, 0:width], lhsT=ones_m[:],
                             rhs=ab_bf[:, 0:width], start=True, stop=True,
                             skip_group_check=True)
            nc.tensor.matmul(ab[:, width:2 * width], lhsT=ones_m[:],
                             rhs=ab_bf[:, width:2 * width], start=True,
                             stop=True, skip_group_check=True)
            return ab

        def ln_apply(ab, width, src_cols, dst, beta_blk, gama_blk,
                     mod_on_act=False, split=False):
            absb = None
            if split:
                absb = trans.tile([128, 2 * width], bf16, tag="absb",
                                  name="absb", bufs=1)
                nc.vector.tensor_copy(out=absb[:], in_=ab[:])
            for d in range(KT):
                on_gp = split and d % 2 == 1
                eng = nc.gpsimd if on_gp else nc.vector
                A = absb if on_gp else ab
                t1 = trans.tile([128, width], bf16, tag="lnt", bufs=4,
                                name="lnt1")
                eng.tensor_tensor(out=t1[:], in0=src_cols(d),
                                  in1=A[:, 0:width], op=OP.mult)
                t2 = trans.tile([128, width], bf16, tag="lnt", bufs=4,
                                name="lnt2")
                eng.tensor_tensor(out=t2[:], in0=t1[:],
                                  in1=A[:, width:2 * width], op=OP.add)
                if mod_on_act:
                    nc.scalar.activation(
                        dst(d), t2[:], AF.Identity,
                        bias=mod_sb[:, gama_blk + d:gama_blk + d + 1],
                        scale=mod1p_sb[:, beta_blk + d:beta_blk + d + 1])
                else:
                    nc.vector.tensor_scalar(
                        out=dst(d), in0=t2[:],
                        scalar1=mod1p_sb[:, beta_blk + d:beta_blk + d + 1],
                        scalar2=mod_sb[:, gama_blk + d:gama_blk + d + 1],
                        op0=OP.mult, op1=OP.add)

        h1T = []
        for d in range(KT):
            h1T.append(reuse.tile([128, SB], bf16, tag=f"rA{d}", bufs=1,
                                  name=f"h1T{d}"))
        ab_h = [ln_prep(ps_st[h], TOK) for h in range(2)]
        for half in range(2):
            c0, c1 = half * TOK, (half + 1) * TOK
            ln_apply(ab_h[half], TOK,
                     lambda d: xfeat_sb[d][:, c0:c1],
                     lambda d: h1T[d][:, c0:c1], 8, 0, mod_on_act=True)

        if DEBUG:
            for d in range(4):
                nc.gpsimd.dma_start(dbg_h1[d], h1T[d][:])
        # ---------- projections ----------
        def evac_proj(dst, ps, bias_i):
            if HAS_COL_BIAS:
                nc.vector.tensor_scalar(out=dst, in0=ps[:],
                                        scalar1=bcol(bias_i), scalar2=None,
                                        op0=OP.add)
            else:
                nc.scalar.activation(dst, ps[:], AF.Copy)

        kT = []
        for oc in range(KT):
            kT.append(reuse.tile([128, SB], bf16, tag=f"rB{oc}", bufs=1,
                                 name=f"kT{oc}"))
        # half-outer so the own-half k projection isn't gated on the
        # other half's modulate; wk slabs stay resident across both halves
        wk_sb = []
        for oc in range(KT):
            wblk = wpool.tile([128, D], bf16, tag="wblk", bufs=8)
            nc.sync.dma_start(wblk[:], wk_t[oc])
            wk_sb.append(wblk)
        for half in range(2):
            c0, c1 = half * TOK, (half + 1) * TOK
            for oc in range(KT):
                ps = psum.tile([128, TOK], f32, tag="proj")
                for k in range(KT):
                    nc.tensor.matmul(
                        ps[:], lhsT=wk_sb[oc][:, k * 128:(k + 1) * 128],
                        rhs=h1T[k][:, c0:c1], start=(k == 0),
                        stop=(k == KT - 1))
                evac_proj(kT[oc][:, c0:c1], ps, 56 + oc)

        if DEBUG:
            for oc in range(KT):
                nc.gpsimd.dma_start(dbg_kT[oc], kT[oc][:])
        # q: own half only
        qT = []
        for oc in range(KT):
            wblk = wpool.tile([128, D], bf16, tag="wblk", bufs=8)
            nc.sync.dma_start(wblk[:], wq_t[oc])
            ps = psum.tile([128, TOK], f32, tag="proj")
            for k in range(KT):
                nc.tensor.matmul(
                    ps[:], lhsT=wblk[:, k * 128:(k + 1) * 128],
                    rhs=h1T[k][:, 0:TOK], start=(k == 0), stop=(k == KT - 1))
            qt = persist.tile([128, TOK], bf16, tag=f"qT{oc}", name=f"qT{oc}")
            evac_proj(qt[:], ps, 48 + oc)
            qT.append(qt)

        if DEBUG:
            for oc in range(KT):
                nc.gpsimd.dma_start(dbg_qT[oc], qT[oc][:])
        # v: token-major over the full batch, evacuated straight into the
        # padded per-head layout [128, head, kc, 65] (col 64 = ones).
        v_h8 = persist.tile([128, 16, 8, 65], bf16, tag="v_h8", name="v_h8")
        nc.vector.memset(v_h8[:, :, :, 64:65], 1.0)
        for fh in range(2):
            wv_tiles = []
            for k in range(KT):
                wblk = wpool.tile([128, TOK], bf16, tag="wvblk", name="wvblk",
                                  bufs=8)
                nc.sync.dma_start(
                    wblk[:], wvT[k * 128:(k + 1) * 128,
                                 fh * 512:(fh + 1) * 512])
                wv_tiles.append(wblk)
            for tb in range(8):
                ps = psum.tile([128, TOK], f32, tag="proj")
                for k in range(KT):
                    nc.tensor.matmul(
                        ps[:], lhsT=h1T[k][:, tb * 128:(tb + 1) * 128],
                        rhs=wv_tiles[k][:],
                        start=(k == 0),
                        stop=(k == KT - 1 and not HAS_ROW_BIAS),
                        skip_group_check=True)
                if HAS_ROW_BIAS:
                    nc.tensor.matmul(
                        ps[:], lhsT=ones_m[:],
                        rhs=bvr_sb[:, fh * 512:(fh + 1) * 512],
                        start=False, stop=True)
                # strided evac: [128, 512] -> heads fh*8..fh*8+7, kc=tb
                nc.scalar.activation(
                    v_h8[:, fh * 8:(fh + 1) * 8, tb, 0:64],
                    ps[:].rearrange("p (h c) -> p h c", h=8), AF.Copy)

        if DEBUG:
            nc.gpsimd.dma_start(dbg_v[:], v_h8[:].rearrange("p a b c -> p (a b c)"))
        # ---------- attention (+ interleaved adaLN blocks 32..47) ----------
        ps_mod2 = psum.tile([128, 32], f32, tag="proj", name="ps_mod2")
        MOD2_PER_HP = [4, 4, 4, 4, 4, 4, 4, 4]
        nmod2 = 0
        attnT = []
        for hp in range(KT):
            attnT.append(persist.tile([128, TOK], bf16, tag=f"attnT{hp}",
                                      name=f"attnT{hp}"))
        SCALE = 1.0 / 8.0

        # Wo weight slabs prefetched; the GEMM itself runs after the loop.
        wo_sb = []
        for dc in range(KT):
            wblk = wpool.tile([128, D], bf16, tag="wblk", bufs=8)
            nc.sync.dma_start(wblk[:], wo_t[dc])
            wo_sb.append(wblk)

        # Schraudolph fast-exp constants for the DVE path (bf16 bit trick):
        # bits16 = x*SCALE*log2(e)*128 + (127*128 - 7.41); rel err ~2% which
        # cancels between numerator and denominator of the softmax.
        EXP_MUL = SCALE * 1.4426950408889634 * 128.0
        EXP_ADD = 16256.0 - 7.41

        for hp in range(KT):
            pT_g = {}
            for g in range(4):
                ps_AB = [psum.tile([128, 2 * TOK], f32, tag="big",
                                   name=f"ps_s{hh}") for hh in range(2)]
                for i in range(2):
                    kc = 2 * g + i
                    for hh in range(2):
                        nc.tensor.matmul(
                            ps_AB[hh][:, i * TOK:(i + 1) * TOK],
                            lhsT=kT[hp][hh * 64:(hh + 1) * 64,
                                        kc * 128:(kc + 1) * 128],
                            rhs=qT[hp][hh * 64:(hh + 1) * 64, :],
                            start=True, stop=True)
                # hh=0 exact exp on ACT; hh=1 fast-exp on DVE
                pt = pT_pool.tile([128, 2 * TOK], bf16, tag="pT",
                                  name="pTg", bufs=5)
                nc.scalar.activation(out=pt[:], in_=ps_AB[0][:],
                                     func=AF.Exp, scale=SCALE)
                pT_g[(0, g)] = pt
                pti = pT_pool.tile([128, 2 * TOK], bf16, tag="pTi",
                                   name="pTi", bufs=5)
                if USE_FAST_EXP:
                    nc.vector.tensor_scalar(
                        out=pti[:].bitcast(mybir.dt.int16), in0=ps_AB[1][:],
                        scalar1=EXP_MUL, scalar2=EXP_ADD,
                        op0=OP.mult, op1=OP.add)
                else:
                    nc.scalar.activation(out=pti[:], in_=ps_AB[1][:],
                                         func=AF.Exp, scale=SCALE)
                pT_g[(1, g)] = pti
            ps_avs = []
            for hh in range(2):
                h = 2 * hp + hh
                ps_av = psum.tile([128, TOK], f32, tag="av")
                for kc in range(8):
                    nc.tensor.matmul(
                        ps_av[0:65, :], lhsT=v_h8[:, h, kc, :],
                        rhs=pT_g[(hh, kc // 2)][:, (kc % 2) * TOK:
                                                (kc % 2 + 1) * TOK],
                        start=(kc == 0), stop=(kc == 7))
                ps_avs.append(ps_av)
                # interleave adaLN blocks 16..47
                for _ in range(MOD2_PER_HP[hp] // 2):
                    if nmod2 < 32:
                        emit_mod_block(ps_mod2, nmod2, 16 + nmod2,
                                       first=(nmod2 == 0), eng=nc.gpsimd)
                        nmod2 += 1
            # per-hp softmax normalization: denominators live in row 64 of
            # each ps_av; stage both heads' reciprocals at partitions 0/64
            # (matmul rhs base must be 0/32/64)
            dn2 = trans.tile([65, 2 * TOK], f32, tag="dn", bufs=1, name="dn2")
            for hh in range(2):
                nc.vector.tensor_copy(out=dn2[64 * hh:64 * hh + 1, 0:TOK],
                                      in_=ps_avs[hh][64:65, :])
            nc.vector.reciprocal_approx_fast(dn2[:, TOK:2 * TOK],
                                             dn2[:, 0:TOK])
            rd2 = trans.tile([65, TOK], bf16, tag="rd", bufs=1, name="rd2")
            for hh in range(2):
                nc.vector.tensor_copy(
                    out=rd2[64 * hh:64 * hh + 1, :],
                    in_=dn2[64 * hh:64 * hh + 1, TOK:2 * TOK])
            for hh in range(2):
                psb = psum.tile([128, TOK], f32, tag="av", name="psb")
                nc.tensor.matmul(psb[0:64, :],
                                 lhsT=ones2[64 * hh:64 * hh + 1, 0:64],
                                 rhs=rd2[64 * hh:64 * hh + 1, :],
                                 start=True, stop=True)
                nc.vector.tensor_copy(out=attnT[hp][hh * 64:(hh + 1) * 64, :],
                                      in_=ps_avs[hh][0:64, :])
                nc.vector.tensor_tensor(
                    out=attnT[hp][hh * 64:(hh + 1) * 64, :],
                    in0=attnT[hp][hh * 64:(hh + 1) * 64, :],
                    in1=psb[0:64, :], op=OP.mult)

        # evacuate adaLN blocks 16..47 (alpha1, gama2, beta2, alpha2)
        if HAS_COL_BIAS:
            nc.vector.tensor_tensor(out=mod_sb[:, 16:48], in0=ps_mod2[:],
                                    in1=bias_sb[:, 16:48], op=OP.add)
        else:
            nc.vector.tensor_copy(out=mod_sb[:, 16:48], in_=ps_mod2[:])
        nc.vector.tensor_scalar_add(mod1p_sb[:, 16:48], mod_sb[:, 16:48], 1.0)

        if DEBUG:
            nc.gpsimd.dma_start(dbg_mod[:], mod_sb[:])
        # ---------- Wo GEMM + epilogue fused with LN2 stats ----------
        # All 64 Wo matmuls first (dense PE stream); the per-dc evac chains
        # (DVE/GpSimd) trail behind; the stats matmuls go after so they
        # don't block the PE queue on the elementwise chain.
        x2T = []
        sq2 = []
        ps_st2 = psum.tile([128, 2 * TOK], f32, tag="big", name="ps_st2")
        for dc in range(KT):
            ps_w = psum.tile([128, TOK], f32, tag="proj", name="ps_wo")
            for hp in range(KT):
                nc.tensor.matmul(ps_w[:],
                                 lhsT=wo_sb[dc][:, hp * 128:(hp + 1) * 128],
                                 rhs=attnT[hp][:], start=(hp == 0),
                                 stop=(hp == KT - 1))
            ysc = trans.tile([128, TOK], f32, tag="sc_evac", name="ysc",
                             bufs=2)
            if HAS_COL_BIAS:
                nc.vector.tensor_scalar(
                    out=ysc[:], in0=ps_w[:], scalar1=bcol(64 + dc),
                    scalar2=mod_sb[:, 16 + dc:17 + dc], op0=OP.add,
                    op1=OP.mult)
            else:
                nc.vector.tensor_scalar(
                    out=ysc[:], in0=ps_w[:],
                    scalar1=mod_sb[:, 16 + dc:17 + dc], scalar2=None,
                    op0=OP.mult)
            x2t = persist.tile([128, TOK], bf16, tag=f"x2T{dc}",
                               name=f"x2T{dc}")
            nc.vector.tensor_tensor(out=x2t[:], in0=ysc[:],
                                    in1=xfeat_sb[dc][:, 0:TOK], op=OP.add)
            x2T.append(x2t)
            sq = trans.tile([128, TOK], bf16, tag="sq2", bufs=6, name="sq2")
            nc.gpsimd.tensor_tensor(out=sq[:], in0=x2t[:], in1=x2t[:],
                                    op=OP.mult)
            sq2.append(sq)
        for dc in range(KT):
            nc.tensor.matmul(ps_st2[0:1, 0:TOK], lhsT=ones_col[:],
                             rhs=x2T[dc][:], start=(dc == 0),
                             stop=(dc == KT - 1), skip_group_check=True)
            nc.tensor.matmul(ps_st2[32:33, TOK:2 * TOK], lhsT=ones_col[:],
                             rhs=sq2[dc][:], start=(dc == 0),
                             stop=(dc == KT - 1), skip_group_check=True)

        h2T = []
        for d in range(KT):
            h2T.append(persist.tile([128, TOK], bf16, tag=f"h2T{d}",
                                    name=f"h2T{d}"))
        ab2 = ln_prep(ps_st2, TOK)
        ps_warm = psum.tile([128, TOK], f32, tag="av", name="ps_warm")
        for i in range(14):
            nc.tensor.matmul(ps_warm[0:1, :], lhsT=ones_col[:],
                             rhs=xfeat_sb[i % 8][:, 0:TOK],
                             start=True, stop=True, skip_group_check=True)
        ln_apply(ab2, TOK, lambda d: x2T[d][:], lambda d: h2T[d][:],
                 32, 24, mod_on_act=True, split=True)

        # ---------- MLP (bf16, token-local) ----------
        G_sb = []
        for g4 in range(8):  # groups of 4 HID blocks
            w1q = wpool.tile([128, 8 * 512], bf16, tag="w1q", bufs=2)
            nc.sync.dma_start(w1q[:], w1_t[g4])
            for jp in range(2):
                ps_g = psum.tile([128, 2 * TOK], f32, tag="big")
                for j2 in range(2):
                    hc = 4 * g4 + 2 * jp + j2
                    korder = [0, 2, 4, 6, 1, 3, 5, 7]
                    for ki, k in enumerate(korder):
                        nc.tensor.matmul(
                            ps_g[:, j2 * TOK:(j2 + 1) * TOK],
                            lhsT=w1q[:, k * 512 + (2 * jp + j2) * 128:
                                     k * 512 + (2 * jp + j2 + 1) * 128],
                            rhs=h2T[k][:],
                            start=(ki == 0), stop=False,
                            skip_group_check=True)
                    if HAS_ROW_BIAS:
                        nc.tensor.matmul(
                            ps_g[:, j2 * TOK:(j2 + 1) * TOK],
                            lhsT=b1r_sb[:, hc * 128:(hc + 1) * 128],
                            rhs=ones_tok[:], start=False, stop=True,
                            skip_group_check=True)
                # reuse the h1T ring (idx 0..7) then the kT ring (8..15);
                # G holds the (2gi | 2gi+1) HID-chunk pair in fp8.
                gi = 2 * g4 + jp
                if gi < 8:
                    gt = reuse.tile([128, SB], bf16, tag=f"rA{gi}", bufs=1,
                                    name=f"G{gi}")
                else:
                    gt = reuse.tile([128, SB], bf16, tag=f"rB{gi - 8}", bufs=1,
                                    name=f"G{gi}")
                nc.scalar.activation(out=gt[:], in_=ps_g[:], func=AF.Gelu)
                G_sb.append(gt)

        out_q = [nc.sync, nc.scalar, nc.gpsimd, nc.scalar]
        for dc in range(KT):
            ps_z = psum.tile([128, TOK], f32, tag="proj")
            for kg2 in range(2):  # two [128, 2048] weight slabs
                wblk = wpool.tile([128, 2048], bf16, tag="w2blk", bufs=2)
                nc.sync.dma_start(wblk[:], w2_t[dc, kg2])
                for i in range(16):
                    kb = 16 * kg2 + i
                    nc.tensor.matmul(
                        ps_z[:], lhsT=wblk[:, i * 128:(i + 1) * 128],
                        rhs=G_sb[kb // 2][:, (kb % 2) * TOK:(kb % 2 + 1) * TOK],
                        start=(kb == 0), stop=(kb == HC - 1))
            zsc = trans.tile([128, TOK], f32, tag="sc_evac", name="zsc",
                             bufs=2)
            if HAS_COL_BIAS:
                nc.vector.tensor_scalar(
                    out=zsc[:], in0=ps_z[:], scalar1=bcol(72 + dc),
                    scalar2=mod_sb[:, 40 + dc:41 + dc],
                    op0=OP.add, op1=OP.mult)
            else:
                nc.vector.tensor_scalar(
                    out=zsc[:], in0=ps_z[:],
                    scalar1=mod_sb[:, 40 + dc:41 + dc], scalar2=None,
                    op0=OP.mult)
            ot = trans.tile([128, TOK], f32, tag="sc_evac", name="ot", bufs=2)
            nc.gpsimd.tensor_tensor(out=ot[:], in0=zsc[:],
                                    in1=x2T[dc][:], op=OP.add)
            out_q[dc % 4].dma_start(out_feat[dc * 128:(dc + 1) * 128, :],
                                    ot[:])

        ctx.close()

    nc.compile()
    return nc


def _pack_bias(bq, bk, bo, b2, bada):
    t = np.zeros((128, 80), np.float32)
    t[:, 0:48] = bada.reshape(48, 128).T
    t[:, 48:56] = bq.reshape(8, 128).T
    t[:, 56:64] = bk.reshape(8, 128).T
    t[:, 64:72] = bo.reshape(8, 128).T
    t[:, 72:80] = b2.reshape(8, 128).T
    return t


def _slab_oc(wT):
    """[D, D] W.T -> [8, 128, 1024]: slab[oc][p][k*128+c] = wT[k*128+p, oc*128+c]"""
    w = wT.reshape(KT, 128, KT, 128)          # [k, p, oc, c]
    return np.ascontiguousarray(w.transpose(2, 1, 0, 3).reshape(KT, 128, D))


WSC = 64.0  # fp8 weight scale (the device folds 1/WSC into the evacuations)


def _fp8(a):
    import ml_dtypes
    return np.clip(a, -240.0, 240.0).astype(ml_dtypes.float8_e4m3)


def _slab_w1(w1T):
    """[D, HID] W1.T -> [8(g4), 128(p), 8(k)*512]: slab[g4][p][k*512+c] =
    w1T[k*128+p, g4*512+c]"""
    w = w1T.reshape(KT, 128, 8, 512)          # [k, p, g4, c]
    return np.ascontiguousarray(w.transpose(2, 1, 0, 3).reshape(KT, 128, 8 * 512))


def _slab_w2(w2T):
    """[HID, D] W2.T -> [8(dc), 2(kg2), 128(p), 16(i)*128]: slab[dc,kg2,p,i*128+c]
    = w2T[(16*kg2+i)*128+p, dc*128+c]"""
    w = w2T.reshape(2, 16, 128, KT, 128)      # [kg2, i, p, dc, c]
    return np.ascontiguousarray(
        w.transpose(3, 0, 2, 1, 4).reshape(KT, 2, 128, 2048))


def _slab_wada(wadaT):
    """[D, 6D] Wada.T -> [48, 128, 1024]: slab[blk][p][k*128+c] =
    wadaT[k*128+p, blk*128+c]"""
    w = wadaT.reshape(KT, 128, 48, 128)       # [k, p, blk, c]
    return np.ascontiguousarray(w.transpose(2, 1, 0, 3).reshape(48, 128, D))


def kernel(x, cond, Wq, bq, Wk, bk, Wv, bv, Wo, bo, W1, b1, W2, b2, Wada, bada):
    import ml_dtypes
    from concourse.bass_utils import run_bass_kernel_spmd

    bf = ml_dtypes.bfloat16
    global HAS_ROW_BIAS, HAS_COL_BIAS
    if "nc" not in _cached:
        HAS_ROW_BIAS = bool(np.any(np.asarray(bv)) or np.any(np.asarray(b1)))
        HAS_COL_BIAS = bool(np.any(np.asarray(bq)) or np.any(np.asarray(bk))
                            or np.any(np.asarray(bo)) or np.any(np.asarray(b2))
                            or np.any(np.asarray(bada)))
        _cached["nc"] = _build()
    nc = _cached["nc"]

    x = np.asarray(x, np.float32)
    cond = np.asarray(cond, np.float32)
    to_bf_T = lambda w: np.ascontiguousarray(
        np.asarray(w, np.float32).T).astype(bf)
    wq_t = _slab_oc(np.asarray(Wq, np.float32).T).astype(bf)
    wk_t = _slab_oc(np.asarray(Wk, np.float32).T).astype(bf)
    wo_t = _slab_oc(np.asarray(Wo, np.float32).T).astype(bf)
    wvT = to_bf_T(Wv)
    w1_t = _slab_w1(np.asarray(W1, np.float32).T).astype(bf)
    w2_t = _slab_w2(np.asarray(W2, np.float32).T).astype(bf)
    wada_b = _slab_wada(np.asarray(Wada, np.float32).T).astype(bf)
    biasc = _pack_bias(np.asarray(bq, np.float32), np.asarray(bk, np.float32),
                       np.asarray(bo, np.float32),
                       np.asarray(b2, np.float32),
                       np.asarray(bada, np.float32))
    bv_row = np.asarray(bv, np.float32).reshape(1, D).astype(bf)

    in_maps = []
    for c in range(N_CORES):
        b, h = c // 2, c % 2
        # own 512 tokens first, then the other half (token-permuted batch)
        xs = np.concatenate([x[b, h * TOK:(h + 1) * TOK, :],
                             x[b, (1 - h) * TOK:(2 - h) * TOK, :]], axis=0)
        in_maps.append({
            "x_feat": np.ascontiguousarray(xs.T).astype(bf),
            "condT": np.ascontiguousarray(cond[b, 0].reshape(8, 128).T),
            "wq_t": wq_t, "wk_t": wk_t, "wo_t": wo_t, "wvT": wvT,
            "w1_t": w1_t, "w2_t": w2_t, "wada_b": wada_b,
            "biasc": biasc, "bv_row": bv_row,
            "b1_row": np.asarray(b1, np.float32).reshape(1, HID).astype(bf),
        })

    _cached["in_maps"] = in_maps
    res = run_bass_kernel_spmd(nc, in_maps, core_ids=list(range(N_CORES)))
    _cached["results"] = res.results
    out = np.empty((B, S, D), np.float32)
    for c in range(N_CORES):
        b, h = c // 2, c % 2
        out[b, h * TOK:(h + 1) * TOK, :] = res.results[c]["out_feat"].T
    return out


# revision 49
# speedup vs baseline: 1.0319x; 1.0055x over previous
"""AdaLN DiT block on 8 Trainium2 NeuronCores — v12, zero collectives.

Sharding: core c owns batch b=c//2 and query-half h=c%2. Host-side the
x tokens are permuted per core so the OWN 512 tokens are always columns
0:512 of x_feat ([D, 1024] feature-major, own|other). Each core computes
LN1 + k/v projections for its FULL batch (1024 tokens) locally. q / Wo /
MLP / residuals are own-half only. No collectives at all.

v7-v12 changes vs v2 (433.5us -> ~391us):
- LN stats matmul pairs col-group-packed: sum in (row 0, bank 0) and
  sumsq in (row 32, bank 1) of one [128, 2*TOK] psum tile. Distinct col
  groups make the pair run concurrently; distinct banks are REQUIRED —
  a start=True bank-clear from one stream lands mid-flight in a
  concurrently-streaming col-tiled matmul sharing the bank and wipes
  its first-touch bits (intermittent negative variance -> NaN).
- ln prep/apply split: per-column a=rstd / b=-mu*rstd rows broadcast
  into one [128, 2*TOK] psum tile; both halves' preps emit before the
  apply chains so the h1 sqrt doesn't head-of-line-block the Scalar
  engine FIFO in front of the projection evacuations.
- Modulates run on the Scalar engine (Identity with AP scale/bias);
  q/k/v PSUM evacuations on Scalar (Copy) — the Vector engine only
  carries the LN mult/add chain in that phase.
- Softmax exp split across engines: head-half 0 exact exp on Scalar,
  head-half 1 via a Schraudolph bf16 bit-trick on Vector (int16 bitcast
  of x*log2(e)*128 + 16248.6; ~2% relative, cancels against the
  denominator computed from the same values).
- Wo is a PSUM-accumulated GEMM after the attention loop (was SBUF f32
  partial accumulation: 64 DVE adds and a ~30us serial tail that let
  HAM re-throttle the PE into fc1). Its LN2 stats matmuls emit after
  the whole GEMM so the PE queue never blocks on the elementwise chain.
- adaLN blocks 0..15 run at the very front (PE warmup during x DMA),
  16..47 interleave evenly into attention hp 0..7 as PE filler.
- Output DMAs spread across 3 queues; x2/sq epilogue split DVE/GpSimd;
  x halves DMA'd on separate queues (gpsimd/scalar) so LN1-h1 unblocks
  ~8us sooner.

PSUM budget (8 banks): tag "big" [128,1024] x2 = 4 banks (stats, ab
broadcasts, scores, fc1), tag "proj" [128,512] x2 = 2 (k/q/v psum,
mod2, Wo, fc2), tag "av" [128,512] x2 = 2 (mod1, AV, psb).
NOTE: matmul start=True clears the WHOLE psum bank; concurrent
(col-tiled) accumulation streams must therefore live in separate banks.
"""

import numpy as np

B, S, D, H, HID = 4, 1024, 1024, 16, 4096
DK = D // H  # 64
N_CORES = 8
TOK = 512    # own tokens per core
SB = 1024    # batch tokens per core (k/v computed locally)
EPS = 1e-6
KT = 8    # 128-row blocks in D
HC = 32   # 128-row blocks in HID

_cached = {}
DEBUG = False
USE_FAST_EXP = True
HAS_ROW_BIAS = False  # bv/b1 nonzero -> adds the rank-1 bias matmuls
HAS_COL_BIAS = False  # bq/bk/bo/b2/bada nonzero -> bias-add epilogues


def _build():
    import contextlib
    import concourse.bass as bass  # noqa: F401
    import concourse.tile as tile
    from concourse import bacc, mybir

    f32 = mybir.dt.float32
    bf16 = mybir.dt.bfloat16
    f8 = mybir.dt.float8e4
    PM = mybir.MatmulPerfMode.DoubleRow
    WSC = 64.0  # host-side fp8 weight scale (descaled in the evacuations)
    ASC = 16.0  # attnT fp8 activation scale (max |attn| <= max |v| ~6)
    AF = mybir.ActivationFunctionType
    OP = mybir.AluOpType

    nc = bacc.Bacc("TRN2", target_bir_lowering=False, debug=False,
                   num_devices=N_CORES)

    # ---- per-core external I/O ----
    x_feat = nc.dram_tensor("x_feat", [D, SB], bf16, kind="ExternalInput")
    condT = nc.dram_tensor("condT", [128, 8], f32, kind="ExternalInput")
    wq_t = nc.dram_tensor("wq_t", [KT, 128, D], bf16, kind="ExternalInput")
    wk_t = nc.dram_tensor("wk_t", [KT, 128, D], bf16, kind="ExternalInput")
    wo_t = nc.dram_tensor("wo_t", [KT, 128, D], bf16, kind="ExternalInput")
    wvT = nc.dram_tensor("wvT", [D, D], bf16, kind="ExternalInput")
    w1_t = nc.dram_tensor("w1_t", [KT, 128, 8 * 512], bf16, kind="ExternalInput")
    w2_t = nc.dram_tensor("w2_t", [KT, 2, 128, 2048], bf16, kind="ExternalInput")
    wada_b = nc.dram_tensor("wada_b", [48, 128, D], bf16, kind="ExternalInput")
    # packed per-partition bias columns (fp32): 0..47 bada, 48..55 bq,
    # 56..63 bk, 64..71 bo, 72..79 b2
    biasc = nc.dram_tensor("biasc", [128, 80], f32, kind="ExternalInput")
    bv_row = nc.dram_tensor("bv_row", [1, D], bf16, kind="ExternalInput")
    b1_row = nc.dram_tensor("b1_row", [1, HID], bf16, kind="ExternalInput")
    out_feat = nc.dram_tensor("out_feat", [D, TOK], f32, kind="ExternalOutput")
    if DEBUG:
        dbg_h1 = nc.dram_tensor("dbg_h1", [4, 128, SB], bf16,
                                kind="ExternalOutput")
        dbg_kT = nc.dram_tensor("dbg_kT", [KT, 128, SB], bf16,
                                kind="ExternalOutput")
        dbg_qT = nc.dram_tensor("dbg_qT", [KT, 128, TOK], bf16,
                                kind="ExternalOutput")
        dbg_v = nc.dram_tensor("dbg_v", [128, 16 * 8 * 65], bf16,
                               kind="ExternalOutput")
        dbg_mod = nc.dram_tensor("dbg_mod", [128, 48], f32,
                                 kind="ExternalOutput")

    with tile.TileContext(nc) as tc:
        ctx = contextlib.ExitStack()
        consts = ctx.enter_context(tc.tile_pool(name="consts", bufs=1))
        persist = ctx.enter_context(tc.tile_pool(name="persist", bufs=1))
        reuse = ctx.enter_context(tc.tile_pool(name="reuse", bufs=1))
        wpool = ctx.enter_context(tc.tile_pool(name="wpool", bufs=3))
        trans = ctx.enter_context(tc.tile_pool(name="trans", bufs=3))
        pT_pool = ctx.enter_context(tc.tile_pool(name="pTp", bufs=2))
        psum = ctx.enter_context(tc.tile_pool(name="psum", bufs=2, space="PSUM"))

        # ---------- constants (cond first — it gates the silu/mod path) ----
        cond_sb = consts.tile([128, 8], f32)
        nc.sync.dma_start(cond_sb[:], condT[:])
        bias_sb = consts.tile([128, 80], f32)
        nc.scalar.dma_start(bias_sb[:], biasc[:])
        if HAS_ROW_BIAS:
            bvr_sb = consts.tile([1, D], bf16)
            nc.scalar.dma_start(bvr_sb[:], bv_row[:])
            b1r_sb = consts.tile([1, HID], bf16)
            nc.scalar.dma_start(b1r_sb[:], b1_row[:])
            ones_tok = consts.tile([1, TOK], bf16)
            nc.vector.memset(ones_tok[:], 1.0)
        ones_col = consts.tile([128, 1], bf16)
        nc.vector.memset(ones_col[:], 1.0)
        eps_sb = consts.tile([1, 1], f32)
        nc.vector.memset(eps_sb[:], EPS)
        ones_m = consts.tile([1, 128], bf16)
        nc.vector.memset(ones_m[:], 1.0)
        ones2 = consts.tile([65, 128], bf16)  # ones rows at partitions 0..64
        nc.vector.memset(ones2[:], 1.0)

        def bcol(i):
            return bias_sb[:, i:i + 1]

        # ---------- adaLN modulation: silu(cond) ----------
        silu_sb = consts.tile([128, 8], bf16)
        nc.scalar.activation(silu_sb[:], cond_sb[:], AF.Silu)

        mod_sb = consts.tile([128, 48], f32)
        mod1p_sb = consts.tile([128, 48], f32)

        def emit_mod_block(ps, col, blk, first, eng=None):
            """One 128-output adaLN block: 256KB DMA + 8 stationary mms."""
            wt = wpool.tile([128, D], bf16, tag="wada", bufs=2, name="wada")
            (eng or nc.sync).dma_start(wt[:], wada_b[blk])
            for k in range(KT):
                nc.tensor.matmul(
                    ps[:, col:col + 1], lhsT=wt[:, k * 128:(k + 1) * 128],
                    rhs=silu_sb[:, k:k + 1],
                    start=(first and k == 0), stop=(k == KT - 1),
                    skip_group_check=True)

        # ---------- phase 1: x DMA + LN1 stats + gama1/beta1 ----------
        xfeat_sb = []
        for d in range(KT):
            xf = persist.tile([128, SB], bf16, tag=f"xfeat{d}", name=f"xf{d}")
            xfeat_sb.append(xf)
        # own-half columns on gpsimd; other half on the scalar queue so
        # both halves land in parallel (~8us earlier h1 -> earlier LN1-h1,
        # k projection). The scalar queue only carries biasc this early.
        for half in range(2):
            for d in range(KT):
                (nc.gpsimd if half == 0 else nc.scalar).dma_start(
                    xfeat_sb[d][:, half * TOK:(half + 1) * TOK],
                    x_feat[d * 128:(d + 1) * 128, half * TOK:(half + 1) * TOK])

        ps_mod = psum.tile([128, 16], f32, tag="av", bufs=2, name="ps_mod")
        # per-half stats tile: sum in row 0, sumsq in row 32 of one bank;
        # the (sum, sumsq) matmul pair runs concurrently via col-groups.
        # sum in (row 0, bank 0), sumsq in (row 32, bank 1): distinct col
        # groups make the pair concurrent, distinct banks make each stream's
        # start=True bank-clear safe against the other.
        ps_st = [psum.tile([128, 2 * TOK], f32, tag="big", name=f"ps_st{h}")
                 for h in range(2)]

        for half in range(2):
            for d in range(KT):
                xs = xfeat_sb[d][:, half * TOK:(half + 1) * TOK]
                sq = trans.tile([128, TOK], bf16, tag="lnsq", bufs=2,
                                name="lnsq")
                nc.vector.tensor_tensor(out=sq[:], in0=xs, in1=xs, op=OP.mult)
                nc.tensor.matmul(ps_st[half][0:1, 0:TOK], lhsT=ones_col[:],
                                 rhs=xs, start=(d == 0), stop=(d == KT - 1),
                                 skip_group_check=True)
                nc.tensor.matmul(ps_st[half][32:33, TOK:2 * TOK],
                                 lhsT=ones_col[:],
                                 rhs=sq[:], start=(d == 0),
                                 stop=(d == KT - 1),
                                 skip_group_check=True)
                blk = 8 * half + d  # gama1 blocks 0..7 then beta1 8..15
                emit_mod_block(ps_mod, blk, blk, first=(blk == 0))

        if HAS_COL_BIAS:
            nc.vector.tensor_tensor(out=mod_sb[:, 0:16], in0=ps_mod[:],
                                    in1=bias_sb[:, 0:16], op=OP.add)
        else:
            nc.vector.tensor_copy(out=mod_sb[:, 0:16], in_=ps_mod[:])
        nc.vector.tensor_scalar_add(mod1p_sb[:, 0:16], mod_sb[:, 0:16], 1.0)

        # ---------- LN finish helpers (prep: stats->broadcast, apply: per-d) --
        def ln_prep(ps_stat, width):
            """ps_stat rows 0(sum)/32(sumsq) -> psum [128, 2w] = [rstd | -mu*rstd]"""
            stA = trans.tile([1, 6 * width], f32, tag="lnstat", name="lnstat",
                             bufs=1)
            mu_n = stA[:, 0:width]             # -mu
            ex2 = stA[:, width:2 * width]
            var = stA[:, 2 * width:3 * width]
            tmp = stA[:, 3 * width:4 * width]  # mu^2 then std
            rstd = stA[:, 4 * width:5 * width]
            bb = stA[:, 5 * width:6 * width]   # -mu*rstd
            nc.vector.tensor_scalar(out=mu_n, in0=ps_stat[0:1, 0:width],
                                    scalar1=-1.0 / D, scalar2=None,
                                    op0=OP.mult)
            nc.vector.tensor_scalar(out=ex2,
                                    in0=ps_stat[32:33, width:2 * width],
                                    scalar1=1.0 / D, scalar2=None,
                                    op0=OP.mult)
            nc.vector.tensor_tensor(out=tmp, in0=mu_n, in1=mu_n, op=OP.mult)
            nc.vector.tensor_tensor(out=var, in0=ex2, in1=tmp, op=OP.subtract)
            nc.scalar.activation(tmp, var, AF.Sqrt, bias=eps_sb[:], scale=1.0)
            nc.vector.reciprocal_approx_fast(rstd, tmp)
            nc.vector.tensor_tensor(out=bb, in0=mu_n, in1=rstd, op=OP.mult)
            ab_bf = trans.tile([1, 2 * width], bf16, tag="lnstatbf",
                               name="lnstatbf", bufs=1)
            nc.vector.tensor_copy(out=ab_bf[:, 0:width], in_=rstd)
            nc.vector.tensor_copy(out=ab_bf[:, width:2 * width], in_=bb)
            ab = psum.tile([128, 2 * width], f32, tag="big", name="ab_bc")
            nc.tensor.matmul(ab[:, 0:width], lhsT=ones_m[:],
                             rhs=ab_bf[:, 0:width], start=True, stop=True,
                             skip_group_check=True)
            nc.tensor.matmul(ab[:, width:2 * width], lhsT=ones_m[:],
                             rhs=ab_bf[:, width:2 * width], start=True,
                             stop=True, skip_group_check=True)
            return ab

        def ln_apply(ab, width, src_cols, dst, beta_blk, gama_blk,
                     mod_on_act=False, split=False):
            absb = None
            if split:
                absb = trans.tile([128, 2 * width], bf16, tag="absb",
                                  name="absb", bufs=1)
                nc.vector.tensor_copy(out=absb[:], in_=ab[:])
            for d in range(KT):
                on_gp = split and d % 2 == 1
                eng = nc.gpsimd if on_gp else nc.vector
                A = absb if on_gp else ab
                t1 = trans.tile([128, width], bf16, tag="lnt", bufs=4,
                                name="lnt1")
                eng.tensor_tensor(out=t1[:], in0=src_cols(d),
                                  in1=A[:, 0:width], op=OP.mult)
                t2 = trans.tile([128, width], bf16, tag="lnt", bufs=4,
                                name="lnt2")
                eng.tensor_tensor(out=t2[:], in0=t1[:],
                                  in1=A[:, width:2 * width], op=OP.add)
                if mod_on_act:
                    nc.scalar.activation(
                        dst(d), t2[:], AF.Identity,
                        bias=mod_sb[:, gama_blk + d:gama_blk + d + 1],
                        scale=mod1p_sb[:, beta_blk + d:beta_blk + d + 1])
                else:
                    nc.vector.tensor_scalar(
                        out=dst(d), in0=t2[:],
                        scalar1=mod1p_sb[:, beta_blk + d:beta_blk + d + 1],
                        scalar2=mod_sb[:, gama_blk + d:gama_blk + d + 1],
                        op0=OP.mult, op1=OP.add)

        h1T = []
        for d in range(KT):
            h1T.append(reuse.tile([128, SB], bf16, tag=f"rA{d}", bufs=1,
                                  name=f"h1T{d}"))
        ab_h = [ln_prep(ps_st[h], TOK) for h in range(2)]
        for half in range(2):
            c0, c1 = half * TOK, (half + 1) * TOK
            ln_apply(ab_h[half], TOK,
                     lambda d: xfeat_sb[d][:, c0:c1],
                     lambda d: h1T[d][:, c0:c1], 8, 0, mod_on_act=True)

        if DEBUG:
            for d in range(4):
                nc.gpsimd.dma_start(dbg_h1[d], h1T[d][:])
        # ---------- projections ----------
        def evac_proj(dst, ps, bias_i):
            if HAS_COL_BIAS:
                nc.vector.tensor_scalar(out=dst, in0=ps[:],
                                        scalar1=bcol(bias_i), scalar2=None,
                                        op0=OP.add)
            else:
                nc.scalar.activation(dst, ps[:], AF.Copy)

        kT = []
        for oc in range(KT):
            kT.append(reuse.tile([128, SB], bf16, tag=f"rB{oc}", bufs=1,
                                 name=f"kT{oc}"))
        # half-outer so the own-half k projection isn't gated on the
        # other half's modulate; wk slabs stay resident across both halves
        wk_sb = []
        for oc in range(KT):
            wblk = wpool.tile([128, D], bf16, tag="wblk", bufs=8)
            nc.sync.dma_start(wblk[:], wk_t[oc])
            wk_sb.append(wblk)
        for half in range(2):
            c0, c1 = half * TOK, (half + 1) * TOK
            for oc in range(KT):
                ps = psum.tile([128, TOK], f32, tag="proj")
                for k in range(KT):
                    nc.tensor.matmul(
                        ps[:], lhsT=wk_sb[oc][:, k * 128:(k + 1) * 128],
                        rhs=h1T[k][:, c0:c1], start=(k == 0),
                        stop=(k == KT - 1))
                evac_proj(kT[oc][:, c0:c1], ps, 56 + oc)

        if DEBUG:
            for oc in range(KT):
                nc.gpsimd.dma_start(dbg_kT[oc], kT[oc][:])
        # q: own half only
        qT = []
        for oc in range(KT):
            wblk = wpool.tile([128, D], bf16, tag="wblk", bufs=8)
            nc.sync.dma_start(wblk[:], wq_t[oc])
            ps = psum.tile([128, TOK], f32, tag="proj")
            for k in range(KT):
                nc.tensor.matmul(
                    ps[:], lhsT=wblk[:, k * 128:(k + 1) * 128],
                    rhs=h1T[k][:, 0:TOK], start=(k == 0), stop=(k == KT - 1))
            qt = persist.tile([128, TOK], bf16, tag=f"qT{oc}", name=f"qT{oc}")
            evac_proj(qt[:], ps, 48 + oc)
            qT.append(qt)

        if DEBUG:
            for oc in range(KT):
                nc.gpsimd.dma_start(dbg_qT[oc], qT[oc][:])
        # v: token-major over the full batch, evacuated straight into the
        # padded per-head layout [128, head, kc, 65] (col 64 = ones).
        v_h8 = persist.tile([128, 16, 8, 65], bf16, tag="v_h8", name="v_h8")
        nc.vector.memset(v_h8[:, :, :, 64:65], 1.0)
        for fh in range(2):
            wv_tiles = []
            for k in range(KT):
                wblk = wpool.tile([128, TOK], bf16, tag="wvblk", name="wvblk",
                                  bufs=8)
                nc.sync.dma_start(
                    wblk[:], wvT[k * 128:(k + 1) * 128,
                                 fh * 512:(fh + 1) * 512])
                wv_tiles.append(wblk)
            for tb in range(8):
                ps = psum.tile([128, TOK], f32, tag="proj")
                for k in range(KT):
                    nc.tensor.matmul(
                        ps[:], lhsT=h1T[k][:, tb * 128:(tb + 1) * 128],
                        rhs=wv_tiles[k][:],
                        start=(k == 0),
                        stop=(k == KT - 1 and not HAS_ROW_BIAS),
                        skip_group_check=True)
                if HAS_ROW_BIAS:
                    nc.tensor.matmul(
                        ps[:], lhsT=ones_m[:],
                        rhs=bvr_sb[:, fh * 512:(fh + 1) * 512],
                        start=False, stop=True)
                # strided evac: [128, 512] -> heads fh*8..fh*8+7, kc=tb
                nc.scalar.activation(
                    v_h8[:, fh * 8:(fh + 1) * 8, tb, 0:64],
                    ps[:].rearrange("p (h c) -> p h c", h=8), AF.Copy)

        if DEBUG:
            nc.gpsimd.dma_start(dbg_v[:], v_h8[:].rearrange("p a b c -> p (a b c)"))
        # ---------- attention (+ interleaved adaLN blocks 32..47) ----------
        ps_mod2 = psum.tile([128, 32], f32, tag="proj", name="ps_mod2")
        MOD2_PER_HP = [4, 4, 4, 4, 4, 4, 4, 4]
        nmod2 = 0
        attnT = []
        for hp in range(KT):
            attnT.append(persist.tile([128, TOK], bf16, tag=f"attnT{hp}",
                                      name=f"attnT{hp}"))
        SCALE = 1.0 / 8.0

        # Wo weight slabs prefetched; the GEMM itself runs after the loop.
        wo_sb = []
        for dc in range(KT):
            wblk = wpool.tile([128, D], bf16, tag="wblk", bufs=8)
            nc.sync.dma_start(wblk[:], wo_t[dc])
            wo_sb.append(wblk)

        # Schraudolph fast-exp constants for the DVE path (bf16 bit trick):
        # bits16 = x*SCALE*log2(e)*128 + (127*128 - 7.41); rel err ~2% which
        # cancels between numerator and denominator of the softmax.
        EXP_MUL = SCALE * 1.4426950408889634 * 128.0
        EXP_ADD = 16256.0 - 7.41

        for hp in range(KT):
            pT_g = {}
            for g in range(4):
                ps_AB = [psum.tile([128, 2 * TOK], f32, tag="big",
                                   name=f"ps_s{hh}") for hh in range(2)]
                for i in range(2):
                    kc = 2 * g + i
                    for hh in range(2):
                        nc.tensor.matmul(
                            ps_AB[hh][:, i * TOK:(i + 1) * TOK],
                            lhsT=kT[hp][hh * 64:(hh + 1) * 64,
                                        kc * 128:(kc + 1) * 128],
                            rhs=qT[hp][hh * 64:(hh + 1) * 64, :],
                            start=True, stop=True)
                # hh=0 exact exp on ACT; hh=1 fast-exp on DVE
                pt = pT_pool.tile([128, 2 * TOK], bf16, tag="pT",
                                  name="pTg", bufs=5)
                nc.scalar.activation(out=pt[:], in_=ps_AB[0][:],
                                     func=AF.Exp, scale=SCALE)
                pT_g[(0, g)] = pt
                pti = pT_pool.tile([128, 2 * TOK], bf16, tag="pTi",
                                   name="pTi", bufs=5)
                if USE_FAST_EXP:
                    nc.vector.tensor_scalar(
                        out=pti[:].bitcast(mybir.dt.int16), in0=ps_AB[1][:],
                        scalar1=EXP_MUL, scalar2=EXP_ADD,
                        op0=OP.mult, op1=OP.add)
                else:
                    nc.scalar.activation(out=pti[:], in_=ps_AB[1][:],
                                         func=AF.Exp, scale=SCALE)
                pT_g[(1, g)] = pti
            ps_avs = []
            for hh in range(2):
                h = 2 * hp + hh
                ps_av = psum.tile([128, TOK], f32, tag="av")
                for kc in range(8):
                    nc.tensor.matmul(
                        ps_av[0:65, :], lhsT=v_h8[:, h, kc, :],
                        rhs=pT_g[(hh, kc // 2)][:, (kc % 2) * TOK:
                                                (kc % 2 + 1) * TOK],
                        start=(kc == 0), stop=(kc == 7))
                ps_avs.append(ps_av)
                # interleave adaLN blocks 16..47
                for _ in range(MOD2_PER_HP[hp] // 2):
                    if nmod2 < 32:
                        emit_mod_block(ps_mod2, nmod2, 16 + nmod2,
                                       first=(nmod2 == 0), eng=nc.gpsimd)
                        nmod2 += 1
            # per-hp softmax normalization: denominators live in row 64 of
            # each ps_av; stage both heads' reciprocals at partitions 0/64
            # (matmul rhs base must be 0/32/64)
            dn2 = trans.tile([65, 2 * TOK], f32, tag="dn", bufs=1, name="dn2")
            for hh in range(2):
                nc.vector.tensor_copy(out=dn2[64 * hh:64 * hh + 1, 0:TOK],
                                      in_=ps_avs[hh][64:65, :])
            nc.vector.reciprocal_approx_fast(dn2[:, TOK:2 * TOK],
                                             dn2[:, 0:TOK])
            rd2 = trans.tile([65, TOK], bf16, tag="rd", bufs=1, name="rd2")
            for hh in range(2):
                nc.vector.tensor_copy(
                    out=rd2[64 * hh:64 * hh + 1, :],
                    in_=dn2[64 * hh:64 * hh + 1, TOK:2 * TOK])
            for hh in range(2):
                psb = psum.tile([128, TOK], f32, tag="av", name="psb")
                nc.tensor.matmul(psb[0:64, :],
                                 lhsT=ones2[64 * hh:64 * hh + 1, 0:64],
                                 rhs=rd2[64 * hh:64 * hh + 1, :],
                                 start=True, stop=True)
                nc.vector.tensor_copy(out=attnT[hp][hh * 64:(hh + 1) * 64, :],
                                      in_=ps_avs[hh][0:64, :])
                nc.vector.tensor_tensor(
                    out=attnT[hp][hh * 64:(hh + 1) * 64, :],
                    in0=attnT[hp][hh * 64:(hh + 1) * 64, :],
                    in1=psb[0:64, :], op=OP.mult)

        # evacuate adaLN blocks 16..47 (alpha1, gama2, beta2, alpha2)
        if HAS_COL_BIAS:
            nc.vector.tensor_tensor(out=mod_sb[:, 16:48], in0=ps_mod2[:],
                                    in1=bias_sb[:, 16:48], op=OP.add)
        else:
            nc.vector.tensor_copy(out=mod_sb[:, 16:48], in_=ps_mod2[:])
        nc.vector.tensor_scalar_add(mod1p_sb[:, 16:48], mod_sb[:, 16:48], 1.0)

        if DEBUG:
            nc.gpsimd.dma_start(dbg_mod[:], mod_sb[:])
        # ---------- Wo GEMM + epilogue fused with LN2 stats ----------
        # All 64 Wo matmuls first (dense PE stream); the per-dc evac chains
        # (DVE/GpSimd) trail behind; the stats matmuls go after so they
        # don't block the PE queue on the elementwise chain.
        x2T = []
        sq2 = []
        ps_st2 = psum.tile([128, 2 * TOK], f32, tag="big", name="ps_st2")
        for dc in range(KT):
            ps_w = psum.tile([128, TOK], f32, tag="proj", name="ps_wo")
            for hp in range(KT):
                nc.tensor.matmul(ps_w[:],
                                 lhsT=wo_sb[dc][:, hp * 128:(hp + 1) * 128],
                                 rhs=attnT[hp][:], start=(hp == 0),
                                 stop=(hp == KT - 1))
            ysc = trans.tile([128, TOK], f32, tag="sc_evac", name="ysc",
                             bufs=2)
            if HAS_COL_BIAS:
                nc.vector.tensor_scalar(
                    out=ysc[:], in0=ps_w[:], scalar1=bcol(64 + dc),
                    scalar2=mod_sb[:, 16 + dc:17 + dc], op0=OP.add,
                    op1=OP.mult)
            else:
                nc.vector.tensor_scalar(
                    out=ysc[:], in0=ps_w[:],
                    scalar1=mod_sb[:, 16 + dc:17 + dc], scalar2=None,
                    op0=OP.mult)
            x2t = persist.tile([128, TOK], bf16, tag=f"x2T{dc}",
                               name=f"x2T{dc}")
            nc.vector.tensor_tensor(out=x2t[:], in0=ysc[:],
                                    in1=xfeat_sb[dc][:, 0:TOK], op=OP.add)
            x2T.append(x2t)
            sq = trans.tile([128, TOK], bf16, tag="sq2", bufs=6, name="sq2")
            nc.gpsimd.tensor_tensor(out=sq[:], in0=x2t[:], in1=x2t[:],
                                    op=OP.mult)
            sq2.append(sq)
        for dc in range(KT):
            nc.tensor.matmul(ps_st2[0:1, 0:TOK], lhsT=ones_col[:],
                             rhs=x2T[dc][:], start=(dc == 0),
                             stop=(dc == KT - 1), skip_group_check=True)
            nc.tensor.matmul(ps_st2[32:33, TOK:2 * TOK], lhsT=ones_col[:],
                             rhs=sq2[dc][:], start=(dc == 0),
                             stop=(dc == KT - 1), skip_group_check=True)

        h2T = []
        for d in range(KT):
            h2T.append(persist.tile([128, TOK], bf16, tag=f"h2T{d}",
                                    name=f"h2T{d}"))
        # warm-keepers split around ln_prep: the first batch covers the PE
        # idle while the DVE stats chain runs (before the broadcast matmuls),
        # the second covers the apply window into fc1.
        ps_warm = psum.tile([128, TOK], f32, tag="av", name="ps_warm")
        for i in range(7):
            nc.tensor.matmul(ps_warm[0:1, :], lhsT=ones_col[:],
                             rhs=xfeat_sb[i % 8][:, 0:TOK],
                             start=True, stop=True, skip_group_check=True)
        ab2 = ln_prep(ps_st2, TOK)
        for i in range(7):
            nc.tensor.matmul(ps_warm[0:1, :], lhsT=ones_col[:],
                             rhs=xfeat_sb[i % 8][:, 0:TOK],
                             start=True, stop=True, skip_group_check=True)
        ln_apply(ab2, TOK, lambda d: x2T[d][:], lambda d: h2T[d][:],
                 32, 24, mod_on_act=True, split=True)

        # ---------- MLP (bf16, token-local) ----------
        G_sb = []
        for g4 in range(8):  # groups of 4 HID blocks
            w1q = wpool.tile([128, 8 * 512], bf16, tag="w1q", bufs=2)
            nc.sync.dma_start(w1q[:], w1_t[g4])
            for jp in range(2):
                ps_g = psum.tile([128, 2 * TOK], f32, tag="big")
                for j2 in range(2):
                    hc = 4 * g4 + 2 * jp + j2
                    korder = [0, 2, 4, 6, 1, 3, 5, 7]
                    for ki, k in enumerate(korder):
                        nc.tensor.matmul(
                            ps_g[:, j2 * TOK:(j2 + 1) * TOK],
                            lhsT=w1q[:, k * 512 + (2 * jp + j2) * 128:
                                     k * 512 + (2 * jp + j2 + 1) * 128],
                            rhs=h2T[k][:],
                            start=(ki == 0), stop=False,
                            skip_group_check=True)
                    if HAS_ROW_BIAS:
                        nc.tensor.matmul(
                            ps_g[:, j2 * TOK:(j2 + 1) * TOK],
                            lhsT=b1r_sb[:, hc * 128:(hc + 1) * 128],
                            rhs=ones_tok[:], start=False, stop=True,
                            skip_group_check=True)
                # reuse the h1T ring (idx 0..7) then the kT ring (8..15);
                # G holds the (2gi | 2gi+1) HID-chunk pair in fp8.
                gi = 2 * g4 + jp
                if gi < 8:
                    gt = reuse.tile([128, SB], bf16, tag=f"rA{gi}", bufs=1,
                                    name=f"G{gi}")
                else:
                    gt = reuse.tile([128, SB], bf16, tag=f"rB{gi - 8}", bufs=1,
                                    name=f"G{gi}")
                nc.scalar.activation(out=gt[:], in_=ps_g[:], func=AF.Gelu)
                G_sb.append(gt)

        out_q = [nc.sync, nc.scalar, nc.gpsimd, nc.scalar]
        for dc in range(KT):
            ps_z = psum.tile([128, TOK], f32, tag="proj")
            for kg2 in range(2):  # two [128, 2048] weight slabs
                wblk = wpool.tile([128, 2048], bf16, tag="w2blk", bufs=2)
                nc.sync.dma_start(wblk[:], w2_t[dc, kg2])
                for i in range(16):
                    kb = 16 * kg2 + i
                    nc.tensor.matmul(
                        ps_z[:], lhsT=wblk[:, i * 128:(i + 1) * 128],
                        rhs=G_sb[kb // 2][:, (kb % 2) * TOK:(kb % 2 + 1) * TOK],
                        start=(kb == 0), stop=(kb == HC - 1))
            zsc = trans.tile([128, TOK], f32, tag="sc_evac", name="zsc",
                             bufs=2)
            if HAS_COL_BIAS:
                nc.vector.tensor_scalar(
                    out=zsc[:], in0=ps_z[:], scalar1=bcol(72 + dc),
                    scalar2=mod_sb[:, 40 + dc:41 + dc],
                    op0=OP.add, op1=OP.mult)
            else:
                nc.vector.tensor_scalar(
                    out=zsc[:], in0=ps_z[:],
                    scalar1=mod_sb[:, 40 + dc:41 + dc], scalar2=None,
                    op0=OP.mult)
            ot = trans.tile([128, TOK], f32, tag="sc_evac", name="ot", bufs=2)
            # last two chunks on the faster DVE: they are pure tail latency
            (nc.vector if dc >= 6 else nc.gpsimd).tensor_tensor(
                out=ot[:], in0=zsc[:], in1=x2T[dc][:], op=OP.add)
            out_q[dc % 4].dma_start(out_feat[dc * 128:(dc + 1) * 128, :],
                                    ot[:])

        ctx.close()

    nc.compile()
    return nc


def _pack_bias(bq, bk, bo, b2, bada):
    t = np.zeros((128, 80), np.float32)
    t[:, 0:48] = bada.reshape(48, 128).T
    t[:, 48:56] = bq.reshape(8, 128).T
    t[:, 56:64] = bk.reshape(8, 128).T
    t[:, 64:72] = bo.reshape(8, 128).T
    t[:, 72:80] = b2.reshape(8, 128).T
    return t


def _slab_oc(wT):
    """[D, D] W.T -> [8, 128, 1024]: slab[oc][p][k*128+c] = wT[k*128+p, oc*128+c]"""
    w = wT.reshape(KT, 128, KT, 128)          # [k, p, oc, c]
    return np.ascontiguousarray(w.transpose(2, 1, 0, 3).reshape(KT, 128, D))


WSC = 64.0  # fp8 weight scale (the device folds 1/WSC into the evacuations)


def _fp8(a):
    import ml_dtypes
    return np.clip(a, -240.0, 240.0).astype(ml_dtypes.float8_e4m3)


def _slab_w1(w1T):
    """[D, HID] W1.T -> [8(g4), 128(p), 8(k)*512]: slab[g4][p][k*512+c] =
    w1T[k*128+p, g4*512+c]"""
    w = w1T.reshape(KT, 128, 8, 512)          # [k, p, g4, c]
    return np.ascontiguousarray(w.transpose(2, 1, 0, 3).reshape(KT, 128, 8 * 512))


def _slab_w2(w2T):
    """[HID, D] W2.T -> [8(dc), 2(kg2), 128(p), 16(i)*128]: slab[dc,kg2,p,i*128+c]
    = w2T[(16*kg2+i)*128+p, dc*128+c]"""
    w = w2T.reshape(2, 16, 128, KT, 128)      # [kg2, i, p, dc, c]
    return np.ascontiguousarray(
        w.transpose(3, 0, 2, 1, 4).reshape(KT, 2, 128, 2048))


def _slab_wada(wadaT):
    """[D, 6D] Wada.T -> [48, 128, 1024]: slab[blk][p][k*128+c] =
    wadaT[k*128+p, blk*128+c]"""
    w = wadaT.reshape(KT, 128, 48, 128)       # [k, p, blk, c]
    return np.ascontiguousarray(w.transpose(2, 1, 0, 3).reshape(48, 128, D))


def kernel(x, cond, Wq, bq, Wk, bk, Wv, bv, Wo, bo, W1, b1, W2, b2, Wada, bada):
    import ml_dtypes
    from concourse.bass_utils import run_bass_kernel_spmd

    bf = ml_dtypes.bfloat16
    global HAS_ROW_BIAS, HAS_COL_BIAS
    if "nc" not in _cached:
        HAS_ROW_BIAS = bool(np.any(np.asarray(bv)) or np.any(np.asarray(b1)))
        HAS_COL_BIAS = bool(np.any(np.asarray(bq)) or np.any(np.asarray(bk))
                            or np.any(np.asarray(bo)) or np.any(np.asarray(b2))
                            or np.any(np.asarray(bada)))
        _cached["nc"] = _build()
    nc = _cached["nc"]

    x = np.asarray(x, np.float32)
    cond = np.asarray(cond, np.float32)
    to_bf_T = lambda w: np.ascontiguousarray(
        np.asarray(w, np.float32).T).astype(bf)
    wq_t = _slab_oc(np.asarray(Wq, np.float32).T).astype(bf)
    wk_t = _slab_oc(np.asarray(Wk, np.float32).T).astype(bf)
    wo_t = _slab_oc(np.asarray(Wo, np.float32).T).astype(bf)
    wvT = to_bf_T(Wv)
    w1_t = _slab_w1(np.asarray(W1, np.float32).T).astype(bf)
    w2_t = _slab_w2(np.asarray(W2, np.float32).T).astype(bf)
    wada_b = _slab_wada(np.asarray(Wada, np.float32).T).astype(bf)
    biasc = _pack_bias(np.asarray(bq, np.float32), np.asarray(bk, np.float32),
                       np.asarray(bo, np.float32),
                       np.asarray(b2, np.float32),
                       np.asarray(bada, np.float32))
    bv_row = np.asarray(bv, np.float32).reshape(1, D).astype(bf)

    in_maps = []
    for c in range(N_CORES):
        b, h = c // 2, c % 2
        # own 512 tokens first, then the other half (token-permuted batch)
        xs = np.concatenate([x[b, h * TOK:(h + 1) * TOK, :],
                             x[b, (1 - h) * TOK:(2 - h) * TOK, :]], axis=0)
        in_maps.append({
            "x_feat": np.ascontiguousarray(xs.T).astype(bf),
            "condT": np.ascontiguousarray(cond[b, 0].reshape(8, 128).T),
            "wq_t": wq_t, "wk_t": wk_t, "wo_t": wo_t, "wvT": wvT,
            "w1_t": w1_t, "w2_t": w2_t, "wada_b": wada_b,
            "biasc": biasc, "bv_row": bv_row,
            "b1_row": np.asarray(b1, np.float32).reshape(1, HID).astype(bf),
        })

    _cached["in_maps"] = in_maps
    res = run_bass_kernel_spmd(nc, in_maps, core_ids=list(range(N_CORES)))
    _cached["results"] = res.results
    out = np.empty((B, S, D), np.float32)
    for c in range(N_CORES):
        b, h = c // 2, c % 2
        out[b, h * TOK:(h + 1) * TOK, :] = res.results[c]["out_feat"].T
    return out
